# revision 1
# baseline (speedup 1.0000x reference)
"""Multi-head Latent Attention (MLA) forward for Trainium2, 8 NeuronCores.

Sharding: core = (batch b in {0,1}) x (head-group of 4 heads out of 16).
Each core computes, for its batch: q/kv down-projections + LayerNorm for
all 2048 tokens (replicated within the batch group), up-projections only
for its 4 heads, causal attention for its 4 heads, and a partial output
projection (contraction over its 512 of 2048 dims). The host sums the 4
partials per batch and adds b_out.

All matmuls run in float32r (full-rate fp32 storage, ~1.5e-4 rel err).
Attention uses a transposed score layout sT[k, t] so softmax needs no
on-chip transposes: exp on ScalarE (no max subtraction; scores are O(1)),
denominators via ones-column matmuls on the TensorEngine, reciprocal
broadcast back to 128 partitions with a K=1 matmul.
"""

import numpy as np

B, S, D, H, HD, L = 2, 2048, 2048, 16, 128, 512
HPC = 4  # heads per core
NCORES = 8
SCALE = 1.0 / np.sqrt(128.0)
EPS = 1e-5
NEG = -1.0e9
NT = S // 128  # 16 token sub-tiles
NHB = S // 256  # 8 half-blocks of 256 tokens
NG = 4  # query-tile groups of 512 tokens
LC = L // 128  # 4 latent chunks
DC = D // 128  # 16 feature chunks

_CACHE = {}
LAST = {}


def _build(has_down_bias, has_ln_affine, has_up_bias, paired=True):
    import contextlib

    import concourse.bass as bass
    import concourse.tile as tile
    from concourse import bacc, mybir
    from concourse.masks import make_identity

    dt = mybir.dt
    f32 = dt.float32
    f32r = dt.float32r
    ACT = mybir.ActivationFunctionType

    nc = bacc.Bacc("TRN2", target_bir_lowering=False, debug=False, num_devices=8)

    def din(name, shape, dtype=None):
        return nc.dram_tensor(
            name, shape, dtype or f32, kind="ExternalInput"
        ).ap()

    x_d = din("x", [S, D], f32r)
    kbias_d = din("kbias", [128, NT])
    wqd_d = din("wqd", [D, L], f32r)
    wkvd_d = din("wkvd", [D, L], f32r)
    wqu_d = din("wqu", [L, HPC * HD], f32r)
    wku_d = din("wku", [L, HPC * HD], f32r)
    wvu_d = din("wvu", [L, HPC * HD], f32r)
    wo_d = din("wo", [HPC * HD, D], f32r)
    if has_down_bias:
        bqd_d = din("bqd", [1, L])
        bkvd_d = din("bkvd", [1, L])
    if has_ln_affine:
        gq_d = din("gq", [1, L])
        bq_d = din("bq", [1, L])
        gkv_d = din("gkv", [1, L])
        bkv_d = din("bkv", [1, L])
    if has_up_bias:
        bqu_d = din("bqu", [128, HPC])  # pre-scaled by SCALE on host
        bku_d = din("bku", [128, HPC])
        bvu_d = din("bvu", [1, HPC * HD])
    out_d = nc.dram_tensor("out", [S, D], f32, kind="ExternalOutput").ap()

    def r(ap):
        return ap.bitcast(f32r)

    with tile.TileContext(nc) as tc:
        with contextlib.ExitStack() as ctx:
            ctx.enter_context(
                nc.allow_low_precision(reason="float32r rounding is intentional")
            )
            const = ctx.enter_context(tc.tile_pool(name="const", bufs=1))

            ident = const.tile([128, 128], f32r, tag="ident")
            ones_col = const.tile([128, 1], f32r, tag="ones_col")
            ones_row = const.tile([1, 128], f32r, tag="ones_row")
            with tc.tile_pool(name="tmpconst", bufs=1) as tmpc:
                ident_f = tmpc.tile([128, 128], f32, tag="ident_f")
                make_identity(nc, ident_f[:])
                nc.vector.tensor_copy(ident[:], ident_f[:])
                ones_f = tmpc.tile([128, 1], f32, tag="ones_f")
                nc.gpsimd.memset(ones_f[:], 1.0)
                nc.vector.tensor_copy(ones_col[:], ones_f[:])
                ones_rf = tmpc.tile([1, 128], f32, tag="ones_rf")
                nc.gpsimd.memset(ones_rf[:], 1.0)
                nc.vector.tensor_copy(ones_row[:], ones_rf[:])

            eps_col = const.tile([128, 1], f32, tag="eps_col")
            nc.gpsimd.memset(eps_col[:], EPS)
            kbias = const.tile([128, NT], f32, tag="kbias")
            nc.sync.dma_start(kbias[:], kbias_d[:])

            if has_ln_affine:
                reps = {}
                for nm, dap in (
                    ("gq", gq_d),
                    ("bq", bq_d),
                    ("gkv", gkv_d),
                    ("bkv", bkv_d),
                ):
                    t = const.tile([128, L], f32, tag=f"rep_{nm}")
                    nc.sync.dma_start(t[:], dap.broadcast_to((128, L)))
                    reps[nm] = t
            if has_down_bias:
                bd_reps = {}
                for nm, dap in (("bqd", bqd_d), ("bkvd", bkvd_d)):
                    t = const.tile([128, L], f32, tag=f"rep_{nm}")
                    nc.sync.dma_start(t[:], dap.broadcast_to((128, L)))
                    bd_reps[nm] = t
            if has_up_bias:
                bqu_sb = const.tile([128, HPC], f32, tag="bqu")
                nc.sync.dma_start(bqu_sb[:], bqu_d[:])
                bku_sb = const.tile([128, HPC], f32, tag="bku")
                nc.sync.dma_start(bku_sb[:], bku_d[:])
                bvu_rep = const.tile([128, HPC * HD], f32, tag="bvu_rep")
                nc.sync.dma_start(bvu_rep[:], bvu_d.broadcast_to((128, HPC * HD)))

            # persistent transposed latents: [128, S] per L-chunk
            latp = ctx.enter_context(tc.tile_pool(name="latT", bufs=1))
            q_latT = [latp.tile([128, S], f32r, tag=f"qlat{c}", name=f"qlat{c}") for c in range(LC)]
            kv_latT = [latp.tile([128, S], f32r, tag=f"kvlat{c}", name=f"kvlat{c}") for c in range(LC)]

            # down-proj weights resident: [128, L] per d-chunk
            p1 = ctx.enter_context(contextlib.ExitStack())
            wpool = p1.enter_context(tc.tile_pool(name="wdown", bufs=1))
            wqd = []
            wkvd = []
            for c in range(DC):
                tq = wpool.tile([128, L], f32r, tag=f"wqd{c}")
                nc.sync.dma_start(tq[:], wqd_d[c * 128 : (c + 1) * 128, :])
                wqd.append(tq)
                tk = wpool.tile([128, L], f32r, tag=f"wkvd{c}")
                nc.sync.dma_start(tk[:], wkvd_d[c * 128 : (c + 1) * 128, :])
                wkvd.append(tk)

            # ------------- Phase 1: x -> xT -> z -> LN -> latT -------------
            xpool = p1.enter_context(tc.tile_pool(name="xin", bufs=3))
            xtpool = p1.enter_context(tc.tile_pool(name="xT", bufs=18))
            zpool = p1.enter_context(tc.tile_pool(name="zpsum", bufs=2, space="PSUM"))
            tpsum = p1.enter_context(tc.tile_pool(name="tpsum", bufs=2, space="PSUM"))
            latsb = p1.enter_context(tc.tile_pool(name="latsb", bufs=4))
            stats = p1.enter_context(tc.tile_pool(name="stats", bufs=8))

            for hb in range(NHB):
                xs = []
                for s in range(2):
                    xt = xpool.tile([128, D], f32r, tag="x")
                    t0 = hb * 256 + s * 128
                    nc.sync.dma_start(xt[:], x_d[t0 : t0 + 128, :])
                    xs.append(xt)
                xT = []
                for c in range(DC):
                    pt = tpsum.tile([128, 256], f32, tag="tp")
                    for s in range(2):
                        nc.tensor.transpose(
                            r(pt[:, s * 128 : (s + 1) * 128]),
                            r(xs[s][:, c * 128 : (c + 1) * 128]),
                            r(ident[:]),
                        )
                    xt = xtpool.tile([128, 256], f32r, tag="xT")
                    if c % 2 == 0:
                        nc.scalar.copy(xt[:], pt[:])
                    else:
                        nc.vector.tensor_copy(xt[:], pt[:])
                    xT.append(xt)

                for s in range(2):
                    zq = zpool.tile([128, L], f32, tag="zq")
                    zkv = zpool.tile([128, L], f32, tag="zkv")
                    for c in range(DC):
                        lhs = r(xT[c][:, s * 128 : (s + 1) * 128])
                        nc.tensor.matmul(
                            zq[:], lhs, r(wqd[c][:]), start=(c == 0), stop=(c == DC - 1)
                        )
                        nc.tensor.matmul(
                            zkv[:],
                            lhs,
                            r(wkvd[c][:]),
                            start=(c == 0),
                            stop=(c == DC - 1),
                        )
                    for path, zp in (("q", zq), ("kv", zkv)):
                        if has_down_bias:
                            zsb = latsb.tile([128, L], f32, tag="zsb")
                            nc.vector.tensor_add(
                                zsb[:],
                                zp[:],
                                bd_reps["bqd" if path == "q" else "bkvd"][:],
                            )
                            zsrc = zsb
                        else:
                            zsrc = zp
                        st6 = stats.tile([128, 6], f32, tag="st6")
                        nc.vector.bn_stats(st6[:], zsrc[:])
                        mv = stats.tile([128, 2], f32, tag="mv")
                        nc.vector.bn_aggr(mv[:], st6[:])
                        mean = mv[:, 0:1]
                        var = mv[:, 1:2]
                        sq = stats.tile([128, 1], f32, tag="sq")
                        nc.scalar.activation(sq[:], var, ACT.Sqrt, bias=eps_col[:], scale=1.0)
                        r0 = stats.tile([128, 1], f32, tag="r0")
                        nc.vector.reciprocal_approx_fast(r0[:], sq[:])
                        u = stats.tile([128, 1], f32, tag="u")
                        nc.vector.tensor_mul(u[:], sq[:], r0[:])
                        u2 = stats.tile([128, 1], f32, tag="u2")
                        nc.vector.tensor_mul(u2[:], u[:], u[:])
                        t3 = stats.tile([128, 1], f32, tag="t3")
                        nc.scalar.activation(t3[:], u2[:], ACT.Copy, bias=1.5, scale=-0.5)
                        rr = stats.tile([128, 1], f32, tag="rr")
                        nc.vector.tensor_mul(rr[:], r0[:], t3[:])
                        nmr = stats.tile([128, 1], f32, tag="nmr")
                        nc.vector.tensor_mul(nmr[:], mean, rr[:])
                        nmr2 = stats.tile([128, 1], f32, tag="nmr2")
                        nc.vector.tensor_scalar_mul(nmr2[:], nmr[:], -1.0)
                        lat = latsb.tile([128, L], f32r, tag="lat")
                        nc.scalar.activation(
                            lat[:], zsrc[:], ACT.Identity, bias=nmr2[:], scale=rr[:]
                        )
                        if has_ln_affine:
                            g_t = reps["gq" if path == "q" else "gkv"]
                            b_t = reps["bq" if path == "q" else "bkv"]
                            lat2 = latsb.tile([128, L], f32r, tag="lat2")
                            nc.vector.tensor_mul(lat2[:], lat[:], g_t[:])
                            lat3 = latsb.tile([128, L], f32r, tag="lat3")
                            nc.vector.tensor_add(lat3[:], lat2[:], b_t[:])
                            lat = lat3
                        dst = q_latT if path == "q" else kv_latT
                        pt = tpsum.tile([128, 512], f32, tag="tpl")
                        for c in range(LC):
                            nc.tensor.transpose(
                                r(pt[:, c * 128 : (c + 1) * 128]),
                                r(lat[:, c * 128 : (c + 1) * 128]),
                                r(ident[:]),
                            )
                        tok0 = hb * 256 + s * 128
                        for c in range(LC):
                            dsub2 = dst[c][:, tok0 : tok0 + 128]
                            psrc = pt[:, c * 128 : (c + 1) * 128]
                            if c % 2 == 0:
                                nc.scalar.copy(dsub2, psrc)
                            else:
                                nc.vector.tensor_copy(dsub2, psrc)

            # ------------- Phase 2: up-projections -------------------------
            p1.close()
            kqv = ctx.enter_context(tc.tile_pool(name="kqv", bufs=1))
            p2 = ctx.enter_context(contextlib.ExitStack())
            upw = p2.enter_context(tc.tile_pool(name="upw", bufs=1))
            w = HPC * HD
            wqu_sb = upw.tile([128, LC * w], f32r, tag="wqu")
            wku_sb = upw.tile([128, LC * w], f32r, tag="wku")
            wvu_sb = upw.tile([128, LC * w], f32r, tag="wvu")
            for c in range(LC):
                nc.sync.dma_start(
                    wqu_sb[:, c * w : (c + 1) * w], wqu_d[c * 128 : (c + 1) * 128, :]
                )
                nc.sync.dma_start(
                    wku_sb[:, c * w : (c + 1) * w], wku_d[c * 128 : (c + 1) * 128, :]
                )
                nc.sync.dma_start(
                    wvu_sb[:, c * w : (c + 1) * w], wvu_d[c * 128 : (c + 1) * 128, :]
                )

            qT = [kqv.tile([128, S], f32r, tag=f"qT{h}", name=f"qT{h}") for h in range(HPC)]
            kT = [kqv.tile([128, S], f32r, tag=f"kT{h}", name=f"kT{h}") for h in range(HPC)]
            vtiles = [kqv.tile([128, w], f32r, tag=f"vt{s}", name=f"vt{s}") for s in range(NT)]
            uppsum = p2.enter_context(tc.tile_pool(name="uppsum", bufs=4, space="PSUM"))

            for G in range(NG):
                for h in range(HPC):
                    for which, wsb, dstT in (("q", wqu_sb, qT), ("k", wku_sb, kT)):
                        pp = uppsum.tile([128, 512], f32, tag="up")
                        for c in range(LC):
                            nc.tensor.matmul(
                                pp[:],
                                r(wsb[:, c * w + h * HD : c * w + (h + 1) * HD]),
                                r(
                                    (q_latT if which == "q" else kv_latT)[c][
                                        :, G * 512 : (G + 1) * 512
                                    ]
                                ),
                                start=(c == 0),
                                stop=(c == LC - 1),
                            )
                        dsub = dstT[h][:, G * 512 : (G + 1) * 512]
                        if has_up_bias:
                            bcol = (bqu_sb if which == "q" else bku_sb)[:, h : h + 1]
                            nc.scalar.activation(
                                dsub,
                                pp[:],
                                ACT.Identity,
                                bias=bcol,
                                scale=SCALE if which == "q" else 1.0,
                            )
                        else:
                            nc.scalar.activation(
                                dsub,
                                pp[:],
                                ACT.Copy,
                                bias=0.0,
                                scale=SCALE if which == "q" else 1.0,
                            )
            for s in range(NT):
                pp = uppsum.tile([128, 512], f32, tag="up")
                for c in range(LC):
                    nc.tensor.matmul(
                        pp[:],
                        r(kv_latT[c][:, s * 128 : (s + 1) * 128]),
                        r(wvu_sb[:, c * w : (c + 1) * w]),
                        start=(c == 0),
                        stop=(c == LC - 1),
                    )
                if has_up_bias:
                    nc.vector.tensor_add(vtiles[s][:], pp[:], bvu_rep[:])
                else:
                    nc.vector.tensor_copy(vtiles[s][:], pp[:])

            # ------------- Phase 3: attention + out-proj -------------------
            p2.close()
            maskp = ctx.enter_context(tc.tile_pool(name="maskp", bufs=1))
            cmask = maskp.tile([128, 128], f32, tag="cmask")
            nc.gpsimd.memset(cmask[:], 0.0)
            # sT[k, t]: keep 0 where (t - k) >= 0, fill NEG where k > t
            nc.gpsimd.affine_select(
                out=cmask[:],
                in_=cmask[:],
                compare_op=mybir.AluOpType.is_ge,
                fill=NEG,
                base=0,
                pattern=[[1, 128]],
                channel_multiplier=-1,
            )
            zeros_r = maskp.tile([128, 384], f32r, tag="zeros_r")
            with tc.tile_pool(name="tmpz", bufs=1) as tmpz:
                zf = tmpz.tile([128, 384], f32, tag="zf")
                nc.gpsimd.memset(zf[:], 0.0)
                nc.vector.tensor_copy(zeros_r[:], zf[:])

            wopool = ctx.enter_context(tc.tile_pool(name="wo", bufs=4))
            spsum = ctx.enter_context(tc.tile_pool(name="spsum", bufs=2, space="PSUM"))
            opsum = ctx.enter_context(tc.tile_pool(name="opsum", bufs=1, space="PSUM"))
            dpsum = ctx.enter_context(tc.tile_pool(name="dpsum", bufs=1, space="PSUM"))
            fpsum = ctx.enter_context(tc.tile_pool(name="fpsum", bufs=2, space="PSUM"))
            expp = ctx.enter_context(tc.tile_pool(name="expp", bufs=2))
            onorm = ctx.enter_context(tc.tile_pool(name="onorm", bufs=5))
            small = ctx.enter_context(tc.tile_pool(name="small", bufs=2))
            outsb = ctx.enter_context(tc.tile_pool(name="outsb", bufs=2))

            def scores_mm(dst, h, G, kc):
                nc.tensor.matmul(
                    dst,
                    r(kT[h][:, kc * 128 : (kc + 1) * 128]),
                    r(qT[h][:, G * 512 : (G + 1) * 512]),
                    start=True,
                    stop=True,
                )

            def den_av(den, otp, es_half, h, G, kc, nkc):
                nc.tensor.matmul(
                    den[:],
                    r(ones_col[:]),
                    es_half,
                    start=(kc == 0),
                    stop=(kc == nkc - 1),
                )
                nc.tensor.matmul(
                    otp[:],
                    r(vtiles[kc][:, h * HD : (h + 1) * HD]),
                    es_half,
                    start=(kc == 0),
                    stop=(kc == nkc - 1),
                )

            for G in range(NG):
                nkc = 4 * G + 4
                otn = []
                for h in range(HPC):
                    otp = fpsum.tile([128, 512], f32, tag="ot")
                    den = dpsum.tile([1, 512], f32, tag="den")
                    # full-width key chunks (below the diagonal band), paired
                    # two per wide psum/exp when the key-padding mask is absent
                    kc = 0
                    while kc < 4 * G:
                        sp = spsum.tile([128, 1024], f32, tag="sc")
                        es = expp.tile([128, 1024], f32r, tag="es")
                        if paired:
                            scores_mm(sp[:, :512], h, G, kc)
                            scores_mm(sp[:, 512:], h, G, kc + 1)
                            nc.scalar.activation(
                                es[:], sp[:], ACT.Exp, bias=0.0, scale=1.0
                            )
                            den_av(den, otp, r(es[:, :512]), h, G, kc, nkc)
                            den_av(den, otp, r(es[:, 512:]), h, G, kc + 1, nkc)
                            kc += 2
                        else:
                            scores_mm(sp[:, :512], h, G, kc)
                            nc.scalar.activation(
                                es[:, :512],
                                sp[:, :512],
                                ACT.Exp,
                                bias=kbias[:, kc : kc + 1],
                                scale=1.0,
                            )
                            den_av(den, otp, r(es[:, :512]), h, G, kc, nkc)
                            kc += 1
                    # diagonal band: causal mask on block j, zeros on dead cols
                    for kc in range(4 * G, nkc):
                        j = kc - 4 * G
                        sp = spsum.tile([128, 1024], f32, tag="sc")
                        es = expp.tile([128, 1024], f32r, tag="es")
                        scores_mm(sp[:, :512], h, G, kc)
                        dsub = slice(j * 128, (j + 1) * 128)
                        nc.vector.tensor_add(sp[:, dsub], sp[:, dsub], cmask[:])
                        if j > 0:
                            nc.vector.tensor_copy(
                                es[:, : j * 128], zeros_r[:, : j * 128]
                            )
                        nc.scalar.activation(
                            es[:, j * 128 : 512],
                            sp[:, j * 128 : 512],
                            ACT.Exp,
                            bias=kbias[:, kc : kc + 1],
                            scale=1.0,
                        )
                        den_av(den, otp, r(es[:, :512]), h, G, kc, nkc)
                    rrow_f = small.tile([1, 512], f32, tag="rrow_f")
                    nc.vector.reciprocal_approx_fast(rrow_f[:], den[:])
                    rrow = small.tile([1, 512], f32r, tag="rrow")
                    nc.vector.tensor_copy(rrow[:], rrow_f[:])
                    rp = spsum.tile([128, 1024], f32, tag="sc", name="rp")
                    nc.tensor.matmul(
                        rp[:, :512], r(ones_row[:]), r(rrow[:]), start=True, stop=True
                    )
                    rep = small.tile([128, 512], f32, tag="rep")
                    nc.scalar.copy(rep[:], rp[:, :512])
                    ot = onorm.tile([128, 512], f32r, tag="otn")
                    nc.vector.tensor_mul(ot[:], otp[:], rep[:])
                    otn.append(ot)

                for jc in range(4):
                    wo_t = []
                    for h in range(HPC):
                        wt = wopool.tile([128, 512], f32r, tag="wo", name=f"wo_{h}")
                        nc.sync.dma_start(
                            wt[:],
                            wo_d[h * 128 : (h + 1) * 128, jc * 512 : (jc + 1) * 512],
                        )
                        wo_t.append(wt)
                    for ls in range(4):
                        op = opsum.tile([128, 512], f32, tag="op")
                        for h in range(HPC):
                            nc.tensor.matmul(
                                op[:],
                                r(otn[h][:, ls * 128 : (ls + 1) * 128]),
                                r(wo_t[h][:]),
                                start=(h == 0),
                                stop=(h == HPC - 1),
                            )
                        ob = outsb.tile([128, 512], f32, tag="ob")
                        nc.scalar.copy(ob[:], op[:])
                        tok0 = G * 512 + ls * 128
                        nc.sync.dma_start(
                            out_d[tok0 : tok0 + 128, jc * 512 : (jc + 1) * 512], ob[:]
                        )

    nc.compile()
    return nc


def kernel(**inputs):
    from concourse.bass_utils import run_bass_kernel_spmd

    x = np.asarray(inputs["x"], np.float32)
    mask = np.asarray(inputs["mask"])
    wq_down = np.ascontiguousarray(np.asarray(inputs["wq_down"], np.float32))
    bq_down = np.asarray(inputs["bq_down"], np.float32)
    gq_ln = np.asarray(inputs["gq_ln"], np.float32)
    bq_ln = np.asarray(inputs["bq_ln"], np.float32)
    wq_up = np.asarray(inputs["wq_up"], np.float32)
    bq_up = np.asarray(inputs["bq_up"], np.float32)
    wkv_down = np.ascontiguousarray(np.asarray(inputs["wkv_down"], np.float32))
    bkv_down = np.asarray(inputs["bkv_down"], np.float32)
    gkv_ln = np.asarray(inputs["gkv_ln"], np.float32)
    bkv_ln = np.asarray(inputs["bkv_ln"], np.float32)
    wkv_up = np.asarray(inputs["wkv_up"], np.float32)
    bkv_up = np.asarray(inputs["bkv_up"], np.float32)
    w_out = np.asarray(inputs["w_out"], np.float32)
    b_out = np.asarray(inputs["b_out"], np.float32)

    has_down_bias = bool(np.any(bq_down) or np.any(bkv_down))
    has_ln_affine = bool(
        np.any(gq_ln != 1.0) or np.any(bq_ln) or np.any(gkv_ln != 1.0) or np.any(bkv_ln)
    )
    has_up_bias = bool(np.any(bq_up) or np.any(bkv_up))
    paired = not bool(np.any(mask))
    key = (has_down_bias, has_ln_affine, has_up_bias, paired)
    if key not in _CACHE:
        _CACHE[key] = _build(*key)
    nc = _CACHE[key]

    wk_up = wkv_up[:, :D]
    wv_up = wkv_up[:, D:]
    bk_up = bkv_up[:D]
    bv_up = bkv_up[D:]

    in_maps = []
    for core in range(NCORES):
        b = core // 4
        g = core % 4
        hs = slice(g * HPC * HD, (g + 1) * HPC * HD)
        kb = np.where(mask[b], np.float32(NEG), np.float32(0.0)).astype(np.float32)
        m = {
            "x": np.ascontiguousarray(x[b]),
            "kbias": np.ascontiguousarray(kb.reshape(NT, 128).T),
            "wqd": wq_down,
            "wkvd": wkv_down,
            "wqu": np.ascontiguousarray(wq_up[:, hs]),
            "wku": np.ascontiguousarray(wk_up[:, hs]),
            "wvu": np.ascontiguousarray(wv_up[:, hs]),
            "wo": np.ascontiguousarray(w_out[hs, :]),
        }
        if has_down_bias:
            m["bqd"] = bq_down.reshape(1, L).copy()
            m["bkvd"] = bkv_down.reshape(1, L).copy()
        if has_ln_affine:
            m["gq"] = gq_ln.reshape(1, L).copy()
            m["bq"] = bq_ln.reshape(1, L).copy()
            m["gkv"] = gkv_ln.reshape(1, L).copy()
            m["bkv"] = bkv_ln.reshape(1, L).copy()
        if has_up_bias:
            m["bqu"] = np.ascontiguousarray(
                (bq_up[hs] * SCALE).reshape(HPC, 128).T.astype(np.float32)
            )
            m["bku"] = np.ascontiguousarray(bk_up[hs].reshape(HPC, 128).T)
            m["bvu"] = np.ascontiguousarray(bv_up[hs].reshape(1, HPC * HD))
        in_maps.append(m)

    import os

    trace = bool(os.environ.get("MLA_TRACE"))
    res = run_bass_kernel_spmd(
        nc, in_maps, core_ids=list(range(NCORES)), trace=trace
    )
    LAST["res"] = res
    partials = np.stack([res.results[i]["out"] for i in range(NCORES)])
    out = partials.reshape(B, 4, S, D).sum(axis=1) + b_out
    return out.astype(np.float32)



# revision 9
# speedup vs baseline: 1.1445x; 1.1445x over previous
"""Multi-head Latent Attention (MLA) forward for Trainium2, 8 NeuronCores.

Sharding: core = (batch b in {0,1}) x (head-group of 4 heads out of 16).
Each core computes, for its batch: q/kv down-projections + LayerNorm for
all 2048 tokens (replicated within the batch group), up-projections only
for its 4 heads, causal attention for its 4 heads, and a partial output
projection (contraction over its 512 of 2048 dims). The host sums the 4
partials per batch and adds b_out.

All matmuls run in float32r (full-rate fp32 storage, ~1.5e-4 rel err).
Attention uses a transposed score layout sT[k, t] so softmax needs no
on-chip transposes: exp on ScalarE (no max subtraction; scores are O(1)),
denominators via ones-column matmuls on the TensorEngine, reciprocal
broadcast back to 128 partitions with a K=1 matmul.
"""

import numpy as np

B, S, D, H, HD, L = 2, 2048, 2048, 16, 128, 512
HPC = 4  # heads per core
NCORES = 8
SCALE = 1.0 / np.sqrt(128.0)
EPS = 1e-5
NEG = -1.0e9
NT = S // 128  # 16 token sub-tiles
NHB = S // 256  # 8 half-blocks of 256 tokens
NG = 4  # query-tile groups of 512 tokens
LC = L // 128  # 4 latent chunks
DC = D // 128  # 16 feature chunks

_CACHE = {}
LAST = {}


def _build(has_down_bias, has_ln_affine, has_up_bias, paired=True):
    import contextlib

    import concourse.bass as bass
    import concourse.tile as tile
    from concourse import bacc, mybir
    from concourse.masks import make_identity

    dt = mybir.dt
    f32 = dt.float32
    f32r = dt.float32r
    ACT = mybir.ActivationFunctionType

    nc = bacc.Bacc("TRN2", target_bir_lowering=False, debug=False, num_devices=8)

    def din(name, shape, dtype=None):
        return nc.dram_tensor(
            name, shape, dtype or f32, kind="ExternalInput"
        ).ap()

    x_d = din("x", [S, D], f32r)
    kbias_d = din("kbias", [128, NT])
    wqd_d = din("wqd", [D, L], f32r)
    wkvd_d = din("wkvd", [D, L], f32r)
    wqu_d = din("wqu", [L, HPC * HD], f32r)
    wku_d = din("wku", [L, HPC * HD], f32r)
    wvu_d = din("wvu", [L, HPC * HD], f32r)
    wo_d = din("wo", [HPC * HD, D], f32r)
    if has_down_bias:
        bqd_d = din("bqd", [1, L])
        bkvd_d = din("bkvd", [1, L])
    if has_ln_affine:
        gq_d = din("gq", [1, L])
        bq_d = din("bq", [1, L])
        gkv_d = din("gkv", [1, L])
        bkv_d = din("bkv", [1, L])
    if has_up_bias:
        bqu_d = din("bqu", [128, HPC])  # pre-scaled by SCALE on host
        bku_d = din("bku", [128, HPC])
        bvu_d = din("bvu", [1, HPC * HD])
    out_d = nc.dram_tensor("out", [S, D], f32, kind="ExternalOutput").ap()

    def r(ap):
        return ap.bitcast(f32r)

    with tile.TileContext(nc) as tc:
        with contextlib.ExitStack() as ctx:
            ctx.enter_context(
                nc.allow_low_precision(reason="float32r rounding is intentional")
            )
            const = ctx.enter_context(tc.tile_pool(name="const", bufs=1))

            ident = const.tile([128, 128], f32r, tag="ident")
            ones_col = const.tile([128, 1], f32r, tag="ones_col")
            ones_row = const.tile([1, 128], f32r, tag="ones_row")
            with tc.tile_pool(name="tmpconst", bufs=1) as tmpc:
                ident_f = tmpc.tile([128, 128], f32, tag="ident_f")
                make_identity(nc, ident_f[:])
                nc.vector.tensor_copy(ident[:], ident_f[:])
                ones_f = tmpc.tile([128, 1], f32, tag="ones_f")
                nc.gpsimd.memset(ones_f[:], 1.0)
                nc.vector.tensor_copy(ones_col[:], ones_f[:])
                ones_rf = tmpc.tile([1, 128], f32, tag="ones_rf")
                nc.gpsimd.memset(ones_rf[:], 1.0)
                nc.vector.tensor_copy(ones_row[:], ones_rf[:])

            eps_col = const.tile([128, 1], f32, tag="eps_col")
            nc.gpsimd.memset(eps_col[:], EPS)
            kbias = const.tile([128, NT], f32, tag="kbias")
            nc.sync.dma_start(kbias[:], kbias_d[:])

            if has_ln_affine:
                reps = {}
                for nm, dap in (
                    ("gq", gq_d),
                    ("bq", bq_d),
                    ("gkv", gkv_d),
                    ("bkv", bkv_d),
                ):
                    t = const.tile([128, L], f32, tag=f"rep_{nm}")
                    nc.sync.dma_start(t[:], dap.broadcast_to((128, L)))
                    reps[nm] = t
            if has_down_bias:
                bd_reps = {}
                for nm, dap in (("bqd", bqd_d), ("bkvd", bkvd_d)):
                    t = const.tile([128, L], f32, tag=f"rep_{nm}")
                    nc.sync.dma_start(t[:], dap.broadcast_to((128, L)))
                    bd_reps[nm] = t
            if has_up_bias:
                bqu_sb = const.tile([128, HPC], f32, tag="bqu")
                nc.sync.dma_start(bqu_sb[:], bqu_d[:])
                bku_sb = const.tile([128, HPC], f32, tag="bku")
                nc.sync.dma_start(bku_sb[:], bku_d[:])
                bvu_rep = const.tile([128, HPC * HD], f32, tag="bvu_rep")
                nc.sync.dma_start(bvu_rep[:], bvu_d.broadcast_to((128, HPC * HD)))

            # persistent transposed latents: [128, S] per L-chunk
            latp = ctx.enter_context(tc.tile_pool(name="latT", bufs=1))
            q_latT = [latp.tile([128, S], f32r, tag=f"qlat{c}", name=f"qlat{c}") for c in range(LC)]
            kv_latT = [latp.tile([128, S], f32r, tag=f"kvlat{c}", name=f"kvlat{c}") for c in range(LC)]

            # pools first so x DMAs for hb=0 can issue before the 8MB of
            # down-proj weights (x arrives in ~3us; weights take ~22us)
            p1 = ctx.enter_context(contextlib.ExitStack())
            wpool = p1.enter_context(tc.tile_pool(name="wdown", bufs=1))
            xpool = p1.enter_context(tc.tile_pool(name="xin", bufs=3))
            xtpool = p1.enter_context(tc.tile_pool(name="xT", bufs=18))
            zpool = p1.enter_context(tc.tile_pool(name="zpsum", bufs=2, space="PSUM"))
            tpsum = p1.enter_context(tc.tile_pool(name="tpsum", bufs=2, space="PSUM"))
            latsb = p1.enter_context(tc.tile_pool(name="latsb", bufs=4))
            stats = p1.enter_context(tc.tile_pool(name="stats", bufs=8))

            xs_pre = []
            for s in range(2):
                xt = xpool.tile([128, D], f32r, tag="x")
                nc.sync.dma_start(xt[:], x_d[s * 128 : (s + 1) * 128, :])
                xs_pre.append(xt)

            # down-proj weights resident: [128, L] per d-chunk
            wqd = []
            wkvd = []
            for c in range(DC):
                tq = wpool.tile([128, L], f32r, tag=f"wqd{c}")
                nc.sync.dma_start(tq[:], wqd_d[c * 128 : (c + 1) * 128, :])
                wqd.append(tq)
                tk = wpool.tile([128, L], f32r, tag=f"wkvd{c}")
                nc.sync.dma_start(tk[:], wkvd_d[c * 128 : (c + 1) * 128, :])
                wkvd.append(tk)

            # ------------- Phase 1: x -> xT -> z -> LN -> latT -------------
            for hb in range(NHB):
                if hb == 0:
                    xs = xs_pre
                else:
                    xs = []
                    for s in range(2):
                        xt = xpool.tile([128, D], f32r, tag="x")
                        t0 = hb * 256 + s * 128
                        nc.sync.dma_start(xt[:], x_d[t0 : t0 + 128, :])
                        xs.append(xt)
                xT = []
                for c in range(DC):
                    pt = tpsum.tile([128, 256], f32, tag="tp")
                    for s in range(2):
                        nc.tensor.transpose(
                            r(pt[:, s * 128 : (s + 1) * 128]),
                            r(xs[s][:, c * 128 : (c + 1) * 128]),
                            r(ident[:]),
                        )
                    xt = xtpool.tile([128, 256], f32r, tag="xT")
                    if c % 2 == 0:
                        nc.scalar.copy(xt[:], pt[:])
                    else:
                        nc.vector.tensor_copy(xt[:], pt[:])
                    xT.append(xt)

                for s in range(2):
                    zq = zpool.tile([128, L], f32, tag="zq")
                    zkv = zpool.tile([128, L], f32, tag="zkv")
                    for c in range(DC):
                        lhs = r(xT[c][:, s * 128 : (s + 1) * 128])
                        nc.tensor.matmul(
                            zq[:], lhs, r(wqd[c][:]), start=(c == 0), stop=(c == DC - 1)
                        )
                        nc.tensor.matmul(
                            zkv[:],
                            lhs,
                            r(wkvd[c][:]),
                            start=(c == 0),
                            stop=(c == DC - 1),
                        )
                    for path, zp in (("q", zq), ("kv", zkv)):
                        if has_down_bias:
                            zsb = latsb.tile([128, L], f32, tag="zsb")
                            nc.vector.tensor_add(
                                zsb[:],
                                zp[:],
                                bd_reps["bqd" if path == "q" else "bkvd"][:],
                            )
                            zsrc = zsb
                        else:
                            zsrc = zp
                        st6 = stats.tile([128, 6], f32, tag="st6")
                        nc.vector.bn_stats(st6[:], zsrc[:])
                        mv = stats.tile([128, 2], f32, tag="mv")
                        nc.vector.bn_aggr(mv[:], st6[:])
                        mean = mv[:, 0:1]
                        var = mv[:, 1:2]
                        sq = stats.tile([128, 1], f32, tag="sq")
                        nc.scalar.activation(sq[:], var, ACT.Sqrt, bias=eps_col[:], scale=1.0)
                        r0 = stats.tile([128, 1], f32, tag="r0")
                        nc.vector.reciprocal_approx_fast(r0[:], sq[:])
                        u = stats.tile([128, 1], f32, tag="u")
                        nc.vector.tensor_mul(u[:], sq[:], r0[:])
                        u2 = stats.tile([128, 1], f32, tag="u2")
                        nc.vector.tensor_mul(u2[:], u[:], u[:])
                        t3 = stats.tile([128, 1], f32, tag="t3")
                        nc.scalar.activation(t3[:], u2[:], ACT.Copy, bias=1.5, scale=-0.5)
                        rr = stats.tile([128, 1], f32, tag="rr")
                        nc.vector.tensor_mul(rr[:], r0[:], t3[:])
                        nmr = stats.tile([128, 1], f32, tag="nmr")
                        nc.vector.tensor_mul(nmr[:], mean, rr[:])
                        nmr2 = stats.tile([128, 1], f32, tag="nmr2")
                        nc.vector.tensor_scalar_mul(nmr2[:], nmr[:], -1.0)
                        lat = latsb.tile([128, L], f32r, tag="lat")
                        nc.scalar.activation(
                            lat[:], zsrc[:], ACT.Identity, bias=nmr2[:], scale=rr[:]
                        )
                        if has_ln_affine:
                            g_t = reps["gq" if path == "q" else "gkv"]
                            b_t = reps["bq" if path == "q" else "bkv"]
                            lat2 = latsb.tile([128, L], f32r, tag="lat2")
                            nc.vector.tensor_mul(lat2[:], lat[:], g_t[:])
                            lat3 = latsb.tile([128, L], f32r, tag="lat3")
                            nc.vector.tensor_add(lat3[:], lat2[:], b_t[:])
                            lat = lat3
                        dst = q_latT if path == "q" else kv_latT
                        pt = tpsum.tile([128, 512], f32, tag="tpl")
                        for c in range(LC):
                            nc.tensor.transpose(
                                r(pt[:, c * 128 : (c + 1) * 128]),
                                r(lat[:, c * 128 : (c + 1) * 128]),
                                r(ident[:]),
                            )
                        tok0 = hb * 256 + s * 128
                        for c in range(LC):
                            dsub2 = dst[c][:, tok0 : tok0 + 128]
                            psrc = pt[:, c * 128 : (c + 1) * 128]
                            if c % 2 == 0:
                                nc.scalar.copy(dsub2, psrc)
                            else:
                                nc.vector.tensor_copy(dsub2, psrc)

            # ------------- Phase 2: up-projections -------------------------
            p1.close()
            kqv = ctx.enter_context(tc.tile_pool(name="kqv", bufs=1))
            p2 = ctx.enter_context(contextlib.ExitStack())
            upw = p2.enter_context(tc.tile_pool(name="upw", bufs=1))
            w = HPC * HD
            wqu_sb = upw.tile([128, LC * w], f32r, tag="wqu")
            wku_sb = upw.tile([128, LC * w], f32r, tag="wku")
            wvu_sb = upw.tile([128, LC * w], f32r, tag="wvu")
            for c in range(LC):
                nc.sync.dma_start(
                    wqu_sb[:, c * w : (c + 1) * w], wqu_d[c * 128 : (c + 1) * 128, :]
                )
                nc.sync.dma_start(
                    wku_sb[:, c * w : (c + 1) * w], wku_d[c * 128 : (c + 1) * 128, :]
                )
                nc.sync.dma_start(
                    wvu_sb[:, c * w : (c + 1) * w], wvu_d[c * 128 : (c + 1) * 128, :]
                )

            qT = [kqv.tile([128, S], f32r, tag=f"qT{h}", name=f"qT{h}") for h in range(HPC)]
            kT = [kqv.tile([128, S], f32r, tag=f"kT{h}", name=f"kT{h}") for h in range(HPC)]
            vtiles = [kqv.tile([128, w], f32r, tag=f"vt{s}", name=f"vt{s}") for s in range(NT)]
            uppsum = p2.enter_context(tc.tile_pool(name="uppsum", bufs=4, space="PSUM"))

            for G in range(NG):
                for h in range(HPC):
                    for which, wsb, dstT in (("q", wqu_sb, qT), ("k", wku_sb, kT)):
                        pp = uppsum.tile([128, 512], f32, tag="up")
                        for c in range(LC):
                            nc.tensor.matmul(
                                pp[:],
                                r(wsb[:, c * w + h * HD : c * w + (h + 1) * HD]),
                                r(
                                    (q_latT if which == "q" else kv_latT)[c][
                                        :, G * 512 : (G + 1) * 512
                                    ]
                                ),
                                start=(c == 0),
                                stop=(c == LC - 1),
                            )
                        dsub = dstT[h][:, G * 512 : (G + 1) * 512]
                        if has_up_bias:
                            bcol = (bqu_sb if which == "q" else bku_sb)[:, h : h + 1]
                            nc.scalar.activation(
                                dsub,
                                pp[:],
                                ACT.Identity,
                                bias=bcol,
                                scale=SCALE if which == "q" else 1.0,
                            )
                        else:
                            nc.scalar.activation(
                                dsub,
                                pp[:],
                                ACT.Copy,
                                bias=0.0,
                                scale=SCALE if which == "q" else 1.0,
                            )
            for s in range(NT):
                pp = uppsum.tile([128, 512], f32, tag="up")
                for c in range(LC):
                    nc.tensor.matmul(
                        pp[:],
                        r(kv_latT[c][:, s * 128 : (s + 1) * 128]),
                        r(wvu_sb[:, c * w : (c + 1) * w]),
                        start=(c == 0),
                        stop=(c == LC - 1),
                    )
                if has_up_bias:
                    nc.vector.tensor_add(vtiles[s][:], pp[:], bvu_rep[:])
                else:
                    nc.vector.tensor_copy(vtiles[s][:], pp[:])

            # ------------- Phase 3: attention + out-proj -------------------
            p2.close()
            # out-proj weights resident: 4MB, loaded once while G=0
            # attention runs (frees 16MB of re-fetch + G-boundary stalls).
            # The latent tiles are dead after phase 2 — reuse them as storage.
            wo_res = q_latT[:HPC]
            for h in range(HPC):
                nc.sync.dma_start(wo_res[h][:], wo_d[h * 128 : (h + 1) * 128, :])
            maskp = ctx.enter_context(tc.tile_pool(name="maskp", bufs=1))
            cmask = maskp.tile([128, 128], f32, tag="cmask")
            nc.gpsimd.memset(cmask[:], 0.0)
            # sT[k, t]: keep 0 where (t - k) >= 0, fill NEG where k > t
            nc.gpsimd.affine_select(
                out=cmask[:],
                in_=cmask[:],
                compare_op=mybir.AluOpType.is_ge,
                fill=NEG,
                base=0,
                pattern=[[1, 128]],
                channel_multiplier=-1,
            )
            zeros_r = maskp.tile([128, 384], f32r, tag="zeros_r")
            with tc.tile_pool(name="tmpz", bufs=1) as tmpz:
                zf = tmpz.tile([128, 384], f32, tag="zf")
                nc.gpsimd.memset(zf[:], 0.0)
                nc.vector.tensor_copy(zeros_r[:], zf[:])

            spsum = ctx.enter_context(tc.tile_pool(name="spsum", bufs=2, space="PSUM"))
            opsum = ctx.enter_context(tc.tile_pool(name="opsum", bufs=1, space="PSUM"))
            dpsum = ctx.enter_context(tc.tile_pool(name="dpsum", bufs=1, space="PSUM"))
            fpsum = ctx.enter_context(tc.tile_pool(name="fpsum", bufs=2, space="PSUM"))
            expp = ctx.enter_context(tc.tile_pool(name="expp", bufs=2))
            onorm = ctx.enter_context(tc.tile_pool(name="onorm", bufs=5))
            small = ctx.enter_context(tc.tile_pool(name="small", bufs=2))
            outsb = ctx.enter_context(tc.tile_pool(name="outsb", bufs=2))
            dsum = ctx.enter_context(tc.tile_pool(name="dsum", bufs=2))

            def scores_mm(dst, h, G, kc):
                nc.tensor.matmul(
                    dst,
                    r(kT[h][:, kc * 128 : (kc + 1) * 128]),
                    r(qT[h][:, G * 512 : (G + 1) * 512]),
                    start=True,
                    stop=True,
                )

            def av_mm(otp, es_half, h, kc, nkc):
                nc.tensor.matmul(
                    otp[:],
                    r(vtiles[kc][:, h * HD : (h + 1) * HD]),
                    es_half,
                    start=(kc == 0),
                    stop=(kc == nkc - 1),
                )

            for G in range(NG):
                nkc = 4 * G + 4
                otn = []
                for h in range(HPC):
                    otp = fpsum.tile([128, 512], f32, tag="ot")
                    # exp-sum accumulator on VectorE (frees ~160 PE matmuls);
                    # one ones-column matmul per (h,G) does the partition
                    # reduction at the end
                    dacc = dsum.tile([128, 512], f32r, tag="dacc")
                    dfirst = True
                    # full-width key chunks (below the diagonal band), paired
                    # two per wide psum/exp when the key-padding mask is absent
                    kc = 0
                    while kc < 4 * G:
                        sp = spsum.tile([128, 1024], f32, tag="sc")
                        es = expp.tile([128, 1024], f32r, tag="es")
                        if paired:
                            scores_mm(sp[:, :512], h, G, kc)
                            scores_mm(sp[:, 512:], h, G, kc + 1)
                            nc.scalar.activation(
                                es[:], sp[:], ACT.Exp, bias=0.0, scale=1.0
                            )
                            av_mm(otp, r(es[:, :512]), h, kc, nkc)
                            av_mm(otp, r(es[:, 512:]), h, kc + 1, nkc)
                            if dfirst:
                                nc.vector.tensor_add(
                                    dacc[:], es[:, :512], es[:, 512:]
                                )
                                dfirst = False
                            else:
                                nc.vector.tensor_add(dacc[:], dacc[:], es[:, :512])
                                nc.vector.tensor_add(dacc[:], dacc[:], es[:, 512:])
                            kc += 2
                        else:
                            scores_mm(sp[:, :512], h, G, kc)
                            nc.scalar.activation(
                                es[:, :512],
                                sp[:, :512],
                                ACT.Exp,
                                bias=kbias[:, kc : kc + 1],
                                scale=1.0,
                            )
                            av_mm(otp, r(es[:, :512]), h, kc, nkc)
                            if dfirst:
                                nc.vector.tensor_copy(dacc[:], es[:, :512])
                                dfirst = False
                            else:
                                nc.vector.tensor_add(dacc[:], dacc[:], es[:, :512])
                            kc += 1
                    # diagonal band: causal mask on block j, zeros on dead cols
                    for kc in range(4 * G, nkc):
                        j = kc - 4 * G
                        sp = spsum.tile([128, 1024], f32, tag="sc")
                        es = expp.tile([128, 1024], f32r, tag="es")
                        scores_mm(sp[:, :512], h, G, kc)
                        dsub = slice(j * 128, (j + 1) * 128)
                        nc.vector.tensor_add(sp[:, dsub], sp[:, dsub], cmask[:])
                        if j > 0:
                            nc.vector.tensor_copy(
                                es[:, : j * 128], zeros_r[:, : j * 128]
                            )
                        nc.scalar.activation(
                            es[:, j * 128 : 512],
                            sp[:, j * 128 : 512],
                            ACT.Exp,
                            bias=kbias[:, kc : kc + 1],
                            scale=1.0,
                        )
                        av_mm(otp, r(es[:, :512]), h, kc, nkc)
                        if dfirst:
                            nc.vector.tensor_copy(dacc[:], es[:, :512])
                            dfirst = False
                        else:
                            nc.vector.tensor_add(dacc[:], dacc[:], es[:, :512])
                    den = dpsum.tile([1, 512], f32, tag="den")
                    nc.tensor.matmul(
                        den[:], r(ones_col[:]), r(dacc[:]), start=True, stop=True
                    )
                    rrow_f = small.tile([1, 512], f32, tag="rrow_f")
                    nc.vector.reciprocal_approx_fast(rrow_f[:], den[:])
                    rrow = small.tile([1, 512], f32r, tag="rrow")
                    nc.vector.tensor_copy(rrow[:], rrow_f[:])
                    rp = spsum.tile([128, 1024], f32, tag="sc", name="rp")
                    nc.tensor.matmul(
                        rp[:, :512], r(ones_row[:]), r(rrow[:]), start=True, stop=True
                    )
                    rep = small.tile([128, 512], f32, tag="rep")
                    nc.scalar.copy(rep[:], rp[:, :512])
                    ot = onorm.tile([128, 512], f32r, tag="otn")
                    nc.vector.tensor_mul(ot[:], otp[:], rep[:])
                    otn.append(ot)

                for jc in range(4):
                    for ls in range(4):
                        op = opsum.tile([128, 512], f32, tag="op")
                        for h in range(HPC):
                            nc.tensor.matmul(
                                op[:],
                                r(otn[h][:, ls * 128 : (ls + 1) * 128]),
                                r(wo_res[h][:, jc * 512 : (jc + 1) * 512]),
                                start=(h == 0),
                                stop=(h == HPC - 1),
                            )
                        ob = outsb.tile([128, 512], f32, tag="ob")
                        nc.scalar.copy(ob[:], op[:])
                        tok0 = G * 512 + ls * 128
                        nc.sync.dma_start(
                            out_d[tok0 : tok0 + 128, jc * 512 : (jc + 1) * 512], ob[:]
                        )

    nc.compile()
    return nc


def kernel(**inputs):
    from concourse.bass_utils import run_bass_kernel_spmd

    x = np.asarray(inputs["x"], np.float32)
    mask = np.asarray(inputs["mask"])
    wq_down = np.ascontiguousarray(np.asarray(inputs["wq_down"], np.float32))
    bq_down = np.asarray(inputs["bq_down"], np.float32)
    gq_ln = np.asarray(inputs["gq_ln"], np.float32)
    bq_ln = np.asarray(inputs["bq_ln"], np.float32)
    wq_up = np.asarray(inputs["wq_up"], np.float32)
    bq_up = np.asarray(inputs["bq_up"], np.float32)
    wkv_down = np.ascontiguousarray(np.asarray(inputs["wkv_down"], np.float32))
    bkv_down = np.asarray(inputs["bkv_down"], np.float32)
    gkv_ln = np.asarray(inputs["gkv_ln"], np.float32)
    bkv_ln = np.asarray(inputs["bkv_ln"], np.float32)
    wkv_up = np.asarray(inputs["wkv_up"], np.float32)
    bkv_up = np.asarray(inputs["bkv_up"], np.float32)
    w_out = np.asarray(inputs["w_out"], np.float32)
    b_out = np.asarray(inputs["b_out"], np.float32)

    has_down_bias = bool(np.any(bq_down) or np.any(bkv_down))
    has_ln_affine = bool(
        np.any(gq_ln != 1.0) or np.any(bq_ln) or np.any(gkv_ln != 1.0) or np.any(bkv_ln)
    )
    has_up_bias = bool(np.any(bq_up) or np.any(bkv_up))
    paired = not bool(np.any(mask))
    key = (has_down_bias, has_ln_affine, has_up_bias, paired)
    if key not in _CACHE:
        _CACHE[key] = _build(*key)
    nc = _CACHE[key]

    wk_up = wkv_up[:, :D]
    wv_up = wkv_up[:, D:]
    bk_up = bkv_up[:D]
    bv_up = bkv_up[D:]

    in_maps = []
    for core in range(NCORES):
        b = core // 4
        g = core % 4
        hs = slice(g * HPC * HD, (g + 1) * HPC * HD)
        kb = np.where(mask[b], np.float32(NEG), np.float32(0.0)).astype(np.float32)
        m = {
            "x": np.ascontiguousarray(x[b]),
            "kbias": np.ascontiguousarray(kb.reshape(NT, 128).T),
            "wqd": wq_down,
            "wkvd": wkv_down,
            "wqu": np.ascontiguousarray(wq_up[:, hs]),
            "wku": np.ascontiguousarray(wk_up[:, hs]),
            "wvu": np.ascontiguousarray(wv_up[:, hs]),
            "wo": np.ascontiguousarray(w_out[hs, :]),
        }
        if has_down_bias:
            m["bqd"] = bq_down.reshape(1, L).copy()
            m["bkvd"] = bkv_down.reshape(1, L).copy()
        if has_ln_affine:
            m["gq"] = gq_ln.reshape(1, L).copy()
            m["bq"] = bq_ln.reshape(1, L).copy()
            m["gkv"] = gkv_ln.reshape(1, L).copy()
            m["bkv"] = bkv_ln.reshape(1, L).copy()
        if has_up_bias:
            m["bqu"] = np.ascontiguousarray(
                (bq_up[hs] * SCALE).reshape(HPC, 128).T.astype(np.float32)
            )
            m["bku"] = np.ascontiguousarray(bk_up[hs].reshape(HPC, 128).T)
            m["bvu"] = np.ascontiguousarray(bv_up[hs].reshape(1, HPC * HD))
        in_maps.append(m)

    import os

    trace = bool(os.environ.get("MLA_TRACE"))
    res = run_bass_kernel_spmd(
        nc, in_maps, core_ids=list(range(NCORES)), trace=trace
    )
    LAST["res"] = res
    partials = np.stack([res.results[i]["out"] for i in range(NCORES)])
    out = partials.reshape(B, 4, S, D).sum(axis=1) + b_out
    return out.astype(np.float32)



# revision 15
# speedup vs baseline: 1.2320x; 1.0765x over previous
"""Multi-head Latent Attention (MLA) forward for Trainium2, 8 NeuronCores.

Sharding: core = (batch b in {0,1}) x (head-group of 4 heads out of 16).
Each core computes, for its batch: q/kv down-projections + LayerNorm for
all 2048 tokens (replicated within the batch group), up-projections only
for its 4 heads, causal attention for its 4 heads, and a partial output
projection (contraction over its 512 of 2048 dims). The host sums the 4
partials per batch and adds b_out.

All matmuls run in float32r (full-rate fp32 storage, ~1.5e-4 rel err).
Attention uses a transposed score layout sT[k, t] so softmax needs no
on-chip transposes: exp on ScalarE (no max subtraction; scores are O(1)),
denominators via ones-column matmuls on the TensorEngine, reciprocal
broadcast back to 128 partitions with a K=1 matmul.
"""

import numpy as np

B, S, D, H, HD, L = 2, 2048, 2048, 16, 128, 512
HPC = 4  # heads per core
NCORES = 8
SCALE = 1.0 / np.sqrt(128.0)
EPS = 1e-5
NEG = -1.0e9
NT = S // 128  # 16 token sub-tiles
NHB = S // 256  # 8 half-blocks of 256 tokens
NG = 4  # query-tile groups of 512 tokens
LC = L // 128  # 4 latent chunks
DC = D // 128  # 16 feature chunks

_CACHE = {}
LAST = {}


def _build(has_down_bias, has_ln_affine, has_up_bias, paired=True):
    import contextlib

    import concourse.bass as bass
    import concourse.tile as tile
    from concourse import bacc, mybir
    from concourse.masks import make_identity

    dt = mybir.dt
    f32 = dt.float32
    f32r = dt.float32r
    ACT = mybir.ActivationFunctionType

    nc = bacc.Bacc("TRN2", target_bir_lowering=False, debug=False, num_devices=8)

    def din(name, shape, dtype=None):
        return nc.dram_tensor(
            name, shape, dtype or f32, kind="ExternalInput"
        ).ap()

    x_d = din("x", [S, D], f32r)
    kbias_d = din("kbias", [128, NT])
    wqd_d = din("wqd", [D, L], f32r)
    wkvd_d = din("wkvd", [D, L], f32r)
    wqu_d = din("wqu", [L, HPC * HD], f32r)
    wku_d = din("wku", [L, HPC * HD], f32r)
    wvu_d = din("wvu", [L, HPC * HD], f32r)
    wo_d = din("wo", [HPC * HD, D], f32r)
    if has_down_bias:
        bqd_d = din("bqd", [1, L])
        bkvd_d = din("bkvd", [1, L])
    if has_ln_affine:
        gq_d = din("gq", [1, L])
        bq_d = din("bq", [1, L])
        gkv_d = din("gkv", [1, L])
        bkv_d = din("bkv", [1, L])
    if has_up_bias:
        bqu_d = din("bqu", [128, HPC])  # pre-scaled by SCALE on host
        bku_d = din("bku", [128, HPC])
        bvu_d = din("bvu", [1, HPC * HD])
    out_d = nc.dram_tensor("out", [S, D], f32, kind="ExternalOutput").ap()

    def r(ap):
        return ap.bitcast(f32r)

    with tile.TileContext(nc) as tc:
        with contextlib.ExitStack() as ctx:
            ctx.enter_context(
                nc.allow_low_precision(reason="float32r rounding is intentional")
            )
            const = ctx.enter_context(tc.tile_pool(name="const", bufs=1))

            ident = const.tile([128, 128], f32r, tag="ident")
            ones_sq = const.tile([128, 128], f32r, tag="ones_sq")
            with tc.tile_pool(name="tmpconst", bufs=1) as tmpc:
                ident_f = tmpc.tile([128, 128], f32, tag="ident_f")
                make_identity(nc, ident_f[:])
                nc.vector.tensor_copy(ident[:], ident_f[:])
                ones_f = tmpc.tile([128, 128], f32, tag="ones_f")
                nc.gpsimd.memset(ones_f[:], 1.0)
                nc.vector.tensor_copy(ones_sq[:], ones_f[:])

            eps_col = const.tile([128, 1], f32, tag="eps_col")
            nc.gpsimd.memset(eps_col[:], EPS)
            kbias = const.tile([128, NT], f32, tag="kbias")
            nc.sync.dma_start(kbias[:], kbias_d[:])

            if has_ln_affine:
                reps = {}
                for nm, dap in (
                    ("gq", gq_d),
                    ("bq", bq_d),
                    ("gkv", gkv_d),
                    ("bkv", bkv_d),
                ):
                    t = const.tile([128, L], f32, tag=f"rep_{nm}")
                    nc.sync.dma_start(t[:], dap.broadcast_to((128, L)))
                    reps[nm] = t
            if has_down_bias:
                bd_reps = {}
                for nm, dap in (("bqd", bqd_d), ("bkvd", bkvd_d)):
                    t = const.tile([128, L], f32, tag=f"rep_{nm}")
                    nc.sync.dma_start(t[:], dap.broadcast_to((128, L)))
                    bd_reps[nm] = t
            if has_up_bias:
                bqu_sb = const.tile([128, HPC], f32, tag="bqu")
                nc.sync.dma_start(bqu_sb[:], bqu_d[:])
                bku_sb = const.tile([128, HPC], f32, tag="bku")
                nc.sync.dma_start(bku_sb[:], bku_d[:])
                bvu_rep = const.tile([128, HPC * HD], f32, tag="bvu_rep")
                nc.sync.dma_start(bvu_rep[:], bvu_d.broadcast_to((128, HPC * HD)))

            # persistent transposed latents: [128, S] per L-chunk
            latp = ctx.enter_context(tc.tile_pool(name="latT", bufs=1))
            q_latT = [latp.tile([128, S], f32r, tag=f"qlat{c}", name=f"qlat{c}") for c in range(LC)]
            kv_latT = [latp.tile([128, S], f32r, tag=f"kvlat{c}", name=f"kvlat{c}") for c in range(LC)]

            # pools first so x DMAs for hb=0 can issue before the 8MB of
            # down-proj weights (x arrives in ~3us; weights take ~22us)
            p1 = ctx.enter_context(contextlib.ExitStack())
            wpool = p1.enter_context(tc.tile_pool(name="wdown", bufs=1))
            xpool = p1.enter_context(tc.tile_pool(name="xin", bufs=3))
            xtpool = p1.enter_context(tc.tile_pool(name="xT", bufs=18))
            zpool = p1.enter_context(tc.tile_pool(name="zpsum", bufs=2, space="PSUM"))
            tpsum = p1.enter_context(tc.tile_pool(name="tpsum", bufs=2, space="PSUM"))
            latsb = p1.enter_context(tc.tile_pool(name="latsb", bufs=4))
            stats = p1.enter_context(tc.tile_pool(name="stats", bufs=8))

            xs_pre = []
            for s in range(2):
                xt = xpool.tile([128, D], f32r, tag="x")
                nc.sync.dma_start(xt[:], x_d[s * 128 : (s + 1) * 128, :])
                xs_pre.append(xt)

            # down-proj weights resident: [128, L] per d-chunk
            wqd = []
            wkvd = []
            for c in range(DC):
                tq = wpool.tile([128, L], f32r, tag=f"wqd{c}")
                nc.sync.dma_start(tq[:], wqd_d[c * 128 : (c + 1) * 128, :])
                wqd.append(tq)
                tk = wpool.tile([128, L], f32r, tag=f"wkvd{c}")
                nc.sync.dma_start(tk[:], wkvd_d[c * 128 : (c + 1) * 128, :])
                wkvd.append(tk)

            # ------------- Phase 1: x -> xT -> z -> LN -> latT -------------
            for hb in range(NHB):
                if hb == 0:
                    xs = xs_pre
                else:
                    xs = []
                    for s in range(2):
                        xt = xpool.tile([128, D], f32r, tag="x")
                        t0 = hb * 256 + s * 128
                        nc.sync.dma_start(xt[:], x_d[t0 : t0 + 128, :])
                        xs.append(xt)
                xT = []
                for c in range(DC):
                    pt = tpsum.tile([128, 256], f32, tag="tp")
                    for s in range(2):
                        nc.tensor.transpose(
                            r(pt[:, s * 128 : (s + 1) * 128]),
                            r(xs[s][:, c * 128 : (c + 1) * 128]),
                            r(ident[:]),
                        )
                    xt = xtpool.tile([128, 256], f32r, tag="xT")
                    if c % 2 == 0:
                        nc.scalar.copy(xt[:], pt[:])
                    else:
                        nc.vector.tensor_copy(xt[:], pt[:])
                    xT.append(xt)

                for s in range(2):
                    zq = zpool.tile([128, L], f32, tag="zq")
                    zkv = zpool.tile([128, L], f32, tag="zkv")
                    for c in range(DC):
                        lhs = r(xT[c][:, s * 128 : (s + 1) * 128])
                        nc.tensor.matmul(
                            zq[:], lhs, r(wqd[c][:]), start=(c == 0), stop=(c == DC - 1)
                        )
                        nc.tensor.matmul(
                            zkv[:],
                            lhs,
                            r(wkvd[c][:]),
                            start=(c == 0),
                            stop=(c == DC - 1),
                        )
                    for path, zp in (("q", zq), ("kv", zkv)):
                        if has_down_bias:
                            zsb = latsb.tile([128, L], f32, tag="zsb")
                            nc.vector.tensor_add(
                                zsb[:],
                                zp[:],
                                bd_reps["bqd" if path == "q" else "bkvd"][:],
                            )
                            zsrc = zsb
                        else:
                            zsrc = zp
                        st6 = stats.tile([128, 6], f32, tag="st6")
                        nc.vector.bn_stats(st6[:], zsrc[:])
                        mv = stats.tile([128, 2], f32, tag="mv")
                        nc.vector.bn_aggr(mv[:], st6[:])
                        mean = mv[:, 0:1]
                        var = mv[:, 1:2]
                        sq = stats.tile([128, 1], f32, tag="sq")
                        nc.scalar.activation(sq[:], var, ACT.Sqrt, bias=eps_col[:], scale=1.0)
                        r0 = stats.tile([128, 1], f32, tag="r0")
                        nc.vector.reciprocal_approx_fast(r0[:], sq[:])
                        u = stats.tile([128, 1], f32, tag="u")
                        nc.vector.tensor_mul(u[:], sq[:], r0[:])
                        u2 = stats.tile([128, 1], f32, tag="u2")
                        nc.vector.tensor_mul(u2[:], u[:], u[:])
                        t3 = stats.tile([128, 1], f32, tag="t3")
                        nc.scalar.activation(t3[:], u2[:], ACT.Copy, bias=1.5, scale=-0.5)
                        rr = stats.tile([128, 1], f32, tag="rr")
                        nc.vector.tensor_mul(rr[:], r0[:], t3[:])
                        nmr = stats.tile([128, 1], f32, tag="nmr")
                        nc.vector.tensor_mul(nmr[:], mean, rr[:])
                        nmr2 = stats.tile([128, 1], f32, tag="nmr2")
                        nc.vector.tensor_scalar_mul(nmr2[:], nmr[:], -1.0)
                        lat = latsb.tile([128, L], f32r, tag="lat")
                        nc.scalar.activation(
                            lat[:], zsrc[:], ACT.Identity, bias=nmr2[:], scale=rr[:]
                        )
                        if has_ln_affine:
                            g_t = reps["gq" if path == "q" else "gkv"]
                            b_t = reps["bq" if path == "q" else "bkv"]
                            lat2 = latsb.tile([128, L], f32r, tag="lat2")
                            nc.vector.tensor_mul(lat2[:], lat[:], g_t[:])
                            lat3 = latsb.tile([128, L], f32r, tag="lat3")
                            nc.vector.tensor_add(lat3[:], lat2[:], b_t[:])
                            lat = lat3
                        dst = q_latT if path == "q" else kv_latT
                        pt = tpsum.tile([128, 512], f32, tag="tpl")
                        for c in range(LC):
                            nc.tensor.transpose(
                                r(pt[:, c * 128 : (c + 1) * 128]),
                                r(lat[:, c * 128 : (c + 1) * 128]),
                                r(ident[:]),
                            )
                        tok0 = hb * 256 + s * 128
                        for c in range(LC):
                            dsub2 = dst[c][:, tok0 : tok0 + 128]
                            psrc = pt[:, c * 128 : (c + 1) * 128]
                            if c % 2 == 0:
                                nc.scalar.copy(dsub2, psrc)
                            else:
                                nc.vector.tensor_copy(dsub2, psrc)

            # ------------- Phase 2: up-projections -------------------------
            p1.close()
            kqv = ctx.enter_context(tc.tile_pool(name="kqv", bufs=1))
            p2 = ctx.enter_context(contextlib.ExitStack())
            upw = p2.enter_context(tc.tile_pool(name="upw", bufs=1))
            w = HPC * HD
            wqu_sb = upw.tile([128, LC * w], f32r, tag="wqu")
            wku_sb = upw.tile([128, LC * w], f32r, tag="wku")
            wvu_sb = upw.tile([128, LC * w], f32r, tag="wvu")
            for c in range(LC):
                nc.sync.dma_start(
                    wqu_sb[:, c * w : (c + 1) * w], wqu_d[c * 128 : (c + 1) * 128, :]
                )
                nc.sync.dma_start(
                    wku_sb[:, c * w : (c + 1) * w], wku_d[c * 128 : (c + 1) * 128, :]
                )
                nc.sync.dma_start(
                    wvu_sb[:, c * w : (c + 1) * w], wvu_d[c * 128 : (c + 1) * 128, :]
                )

            qT = [kqv.tile([128, S], f32r, tag=f"qT{h}", name=f"qT{h}") for h in range(HPC)]
            kT = [kqv.tile([128, S], f32r, tag=f"kT{h}", name=f"kT{h}") for h in range(HPC)]
            vtiles = [kqv.tile([128, w], f32r, tag=f"vt{s}", name=f"vt{s}") for s in range(NT)]
            uppsum = p2.enter_context(tc.tile_pool(name="uppsum", bufs=4, space="PSUM"))

            for G in range(NG):
                for h in range(HPC):
                    for which, wsb, dstT in (("q", wqu_sb, qT), ("k", wku_sb, kT)):
                        pp = uppsum.tile([128, 512], f32, tag="up")
                        for c in range(LC):
                            nc.tensor.matmul(
                                pp[:],
                                r(wsb[:, c * w + h * HD : c * w + (h + 1) * HD]),
                                r(
                                    (q_latT if which == "q" else kv_latT)[c][
                                        :, G * 512 : (G + 1) * 512
                                    ]
                                ),
                                start=(c == 0),
                                stop=(c == LC - 1),
                            )
                        dsub = dstT[h][:, G * 512 : (G + 1) * 512]
                        if has_up_bias:
                            bcol = (bqu_sb if which == "q" else bku_sb)[:, h : h + 1]
                            nc.scalar.activation(
                                dsub,
                                pp[:],
                                ACT.Identity,
                                bias=bcol,
                                scale=SCALE if which == "q" else 1.0,
                            )
                        else:
                            nc.scalar.activation(
                                dsub,
                                pp[:],
                                ACT.Copy,
                                bias=0.0,
                                scale=SCALE if which == "q" else 1.0,
                            )
            for s in range(NT):
                pp = uppsum.tile([128, 512], f32, tag="up")
                for c in range(LC):
                    nc.tensor.matmul(
                        pp[:],
                        r(kv_latT[c][:, s * 128 : (s + 1) * 128]),
                        r(wvu_sb[:, c * w : (c + 1) * w]),
                        start=(c == 0),
                        stop=(c == LC - 1),
                    )
                if has_up_bias:
                    nc.vector.tensor_add(vtiles[s][:], pp[:], bvu_rep[:])
                else:
                    nc.vector.tensor_copy(vtiles[s][:], pp[:])

            # ------------- Phase 3: attention + out-proj -------------------
            p2.close()
            # out-proj weights resident: 4MB, loaded once while G=0
            # attention runs (frees 16MB of re-fetch + G-boundary stalls).
            # The latent tiles are dead after phase 2 — reuse them as storage.
            wo_res = q_latT[:HPC]
            for h in range(HPC):
                nc.sync.dma_start(wo_res[h][:], wo_d[h * 128 : (h + 1) * 128, :])
            maskp = ctx.enter_context(tc.tile_pool(name="maskp", bufs=1))
            cmask = maskp.tile([128, 128], f32, tag="cmask")
            nc.gpsimd.memset(cmask[:], 0.0)
            # sT[k, t]: keep 0 where (t - k) >= 0, fill NEG where k > t
            nc.gpsimd.affine_select(
                out=cmask[:],
                in_=cmask[:],
                compare_op=mybir.AluOpType.is_ge,
                fill=NEG,
                base=0,
                pattern=[[1, 128]],
                channel_multiplier=-1,
            )
            zeros_r = maskp.tile([128, 384], f32r, tag="zeros_r")
            with tc.tile_pool(name="tmpz", bufs=1) as tmpz:
                zf = tmpz.tile([128, 384], f32, tag="zf")
                nc.gpsimd.memset(zf[:], 0.0)
                nc.vector.tensor_copy(zeros_r[:], zf[:])

            spsum = ctx.enter_context(tc.tile_pool(name="spsum", bufs=2, space="PSUM"))
            opsum = ctx.enter_context(tc.tile_pool(name="opsum", bufs=2, space="PSUM"))
            dpsum = ctx.enter_context(tc.tile_pool(name="dpsum", bufs=2, space="PSUM"))
            fpsum = ctx.enter_context(tc.tile_pool(name="fpsum", bufs=2, space="PSUM"))
            expp = ctx.enter_context(tc.tile_pool(name="expp", bufs=3))
            onorm = ctx.enter_context(tc.tile_pool(name="onorm", bufs=5))
            small = ctx.enter_context(tc.tile_pool(name="small", bufs=3))
            outsb = ctx.enter_context(tc.tile_pool(name="outsb", bufs=2))
            dsum = ctx.enter_context(tc.tile_pool(name="dsum", bufs=3))

            def scores_mm(dst, h, G, kc):
                nc.tensor.matmul(
                    dst,
                    r(kT[h][:, kc * 128 : (kc + 1) * 128]),
                    r(qT[h][:, G * 512 : (G + 1) * 512]),
                    start=True,
                    stop=True,
                )

            def av_mm(otp, es_half, h, kc, nkc):
                nc.tensor.matmul(
                    otp[:],
                    r(vtiles[kc][:, h * HD : (h + 1) * HD]),
                    es_half,
                    start=(kc == 0),
                    stop=(kc == nkc - 1),
                )

            for G in range(NG):
                nkc = 4 * G + 4
                # exp-sums accumulate on VectorE per head (frees ~160 PE
                # matmuls). The partition reduction uses an all-ones 128x128
                # stationary, so its [128,512] output IS the broadcast of the
                # denominator to every partition: reciprocal runs full-width
                # on VectorE and the whole normalize tail stays off the PE.
                otn = []
                for h in range(HPC):
                    otp = fpsum.tile([128, 512], f32, tag="ot")
                    dacc = dsum.tile([128, 512], f32r, tag="dacc")
                    for kc in range(nkc):
                        j = kc - 4 * G
                        sp = spsum.tile([128, 512], f32, tag="sc")
                        es = expp.tile([128, 512], f32r, tag="es")
                        if j <= 0:
                            # full-width key chunk (at/below the diagonal)
                            scores_mm(sp[:], h, G, kc)
                        else:
                            # diagonal band: queries < j*128 are fully masked
                            nc.tensor.matmul(
                                sp[:, j * 128 :],
                                r(kT[h][:, kc * 128 : (kc + 1) * 128]),
                                r(qT[h][:, G * 512 + j * 128 : (G + 1) * 512]),
                                start=True,
                                stop=True,
                            )
                        if j < 0:
                            if paired:
                                nc.scalar.activation(
                                    es[:], sp[:], ACT.Exp, bias=0.0, scale=1.0
                                )
                            else:
                                nc.scalar.activation(
                                    es[:],
                                    sp[:],
                                    ACT.Exp,
                                    bias=kbias[:, kc : kc + 1],
                                    scale=1.0,
                                )
                        else:
                            # causal mask on block j, zeros on dead columns
                            dsub = slice(j * 128, (j + 1) * 128)
                            nc.vector.tensor_add(sp[:, dsub], sp[:, dsub], cmask[:])
                            if j > 0:
                                nc.vector.tensor_copy(
                                    es[:, : j * 128], zeros_r[:, : j * 128]
                                )
                            nc.scalar.activation(
                                es[:, j * 128 :],
                                sp[:, j * 128 :],
                                ACT.Exp,
                                bias=kbias[:, kc : kc + 1],
                                scale=1.0,
                            )
                        av_mm(otp, r(es[:]), h, kc, nkc)
                        if kc == 0:
                            nc.vector.tensor_copy(dacc[:], es[:])
                        else:
                            nc.vector.tensor_add(dacc[:], dacc[:], es[:])
                    denb = dpsum.tile([128, 512], f32, tag="denb")
                    nc.tensor.matmul(
                        denb[:], r(ones_sq[:]), r(dacc[:]), start=True, stop=True
                    )
                    rep = small.tile([128, 512], f32, tag="rep")
                    nc.vector.reciprocal_approx_fast(rep[:], denb[:])
                    ot = onorm.tile([128, 512], f32r, tag="otn")
                    nc.vector.tensor_mul(ot[:], otp[:], rep[:])
                    otn.append(ot)

                for jc in range(4):
                    for ls in range(4):
                        op = opsum.tile([128, 512], f32, tag="op")
                        for h in range(HPC):
                            nc.tensor.matmul(
                                op[:],
                                r(otn[h][:, ls * 128 : (ls + 1) * 128]),
                                r(wo_res[h][:, jc * 512 : (jc + 1) * 512]),
                                start=(h == 0),
                                stop=(h == HPC - 1),
                            )
                        ob = outsb.tile([128, 512], f32, tag="ob")
                        if ls % 2 == 0:
                            nc.scalar.copy(ob[:], op[:])
                        else:
                            nc.vector.tensor_copy(ob[:], op[:])
                        tok0 = G * 512 + ls * 128
                        nc.sync.dma_start(
                            out_d[tok0 : tok0 + 128, jc * 512 : (jc + 1) * 512], ob[:]
                        )

    nc.compile()
    return nc


def kernel(**inputs):
    from concourse.bass_utils import run_bass_kernel_spmd

    x = np.asarray(inputs["x"], np.float32)
    mask = np.asarray(inputs["mask"])
    wq_down = np.ascontiguousarray(np.asarray(inputs["wq_down"], np.float32))
    bq_down = np.asarray(inputs["bq_down"], np.float32)
    gq_ln = np.asarray(inputs["gq_ln"], np.float32)
    bq_ln = np.asarray(inputs["bq_ln"], np.float32)
    wq_up = np.asarray(inputs["wq_up"], np.float32)
    bq_up = np.asarray(inputs["bq_up"], np.float32)
    wkv_down = np.ascontiguousarray(np.asarray(inputs["wkv_down"], np.float32))
    bkv_down = np.asarray(inputs["bkv_down"], np.float32)
    gkv_ln = np.asarray(inputs["gkv_ln"], np.float32)
    bkv_ln = np.asarray(inputs["bkv_ln"], np.float32)
    wkv_up = np.asarray(inputs["wkv_up"], np.float32)
    bkv_up = np.asarray(inputs["bkv_up"], np.float32)
    w_out = np.asarray(inputs["w_out"], np.float32)
    b_out = np.asarray(inputs["b_out"], np.float32)

    has_down_bias = bool(np.any(bq_down) or np.any(bkv_down))
    has_ln_affine = bool(
        np.any(gq_ln != 1.0) or np.any(bq_ln) or np.any(gkv_ln != 1.0) or np.any(bkv_ln)
    )
    has_up_bias = bool(np.any(bq_up) or np.any(bkv_up))
    paired = not bool(np.any(mask))
    key = (has_down_bias, has_ln_affine, has_up_bias, paired)
    if key not in _CACHE:
        _CACHE[key] = _build(*key)
    nc = _CACHE[key]

    wk_up = wkv_up[:, :D]
    wv_up = wkv_up[:, D:]
    bk_up = bkv_up[:D]
    bv_up = bkv_up[D:]

    in_maps = []
    for core in range(NCORES):
        b = core // 4
        g = core % 4
        hs = slice(g * HPC * HD, (g + 1) * HPC * HD)
        kb = np.where(mask[b], np.float32(NEG), np.float32(0.0)).astype(np.float32)
        m = {
            "x": np.ascontiguousarray(x[b]),
            "kbias": np.ascontiguousarray(kb.reshape(NT, 128).T),
            "wqd": wq_down,
            "wkvd": wkv_down,
            "wqu": np.ascontiguousarray(wq_up[:, hs]),
            "wku": np.ascontiguousarray(wk_up[:, hs]),
            "wvu": np.ascontiguousarray(wv_up[:, hs]),
            "wo": np.ascontiguousarray(w_out[hs, :]),
        }
        if has_down_bias:
            m["bqd"] = bq_down.reshape(1, L).copy()
            m["bkvd"] = bkv_down.reshape(1, L).copy()
        if has_ln_affine:
            m["gq"] = gq_ln.reshape(1, L).copy()
            m["bq"] = bq_ln.reshape(1, L).copy()
            m["gkv"] = gkv_ln.reshape(1, L).copy()
            m["bkv"] = bkv_ln.reshape(1, L).copy()
        if has_up_bias:
            m["bqu"] = np.ascontiguousarray(
                (bq_up[hs] * SCALE).reshape(HPC, 128).T.astype(np.float32)
            )
            m["bku"] = np.ascontiguousarray(bk_up[hs].reshape(HPC, 128).T)
            m["bvu"] = np.ascontiguousarray(bv_up[hs].reshape(1, HPC * HD))
        in_maps.append(m)

    import os

    trace = bool(os.environ.get("MLA_TRACE"))
    res = run_bass_kernel_spmd(
        nc, in_maps, core_ids=list(range(NCORES)), trace=trace
    )
    LAST["res"] = res
    partials = np.stack([res.results[i]["out"] for i in range(NCORES)])
    out = partials.reshape(B, 4, S, D).sum(axis=1) + b_out
    return out.astype(np.float32)



# revision 27
# speedup vs baseline: 1.3838x; 1.1232x over previous
"""Multi-head Latent Attention (MLA) forward for Trainium2, 8 NeuronCores.

Sharding: core = (batch b in {0,1}) x (head-group of 4 heads out of 16).
Each core computes, for its batch: q/kv down-projections + LayerNorm for
all 2048 tokens (replicated within the batch group), up-projections only
for its 4 heads, causal attention for its 4 heads, and a partial output
projection (contraction over its 512 of 2048 dims). The host sums the 4
partials per batch and adds b_out.

All matmuls run in float32r (full-rate fp32 storage, ~1.5e-4 rel err).
Attention uses a transposed score layout sT[k, t] so softmax needs no
on-chip transposes: exp on ScalarE (no max subtraction; scores are O(1)),
denominators via ones-column matmuls on the TensorEngine, reciprocal
broadcast back to 128 partitions with a K=1 matmul.
"""

import numpy as np

B, S, D, H, HD, L = 2, 2048, 2048, 16, 128, 512
HPC = 4  # heads per core
NCORES = 8
SCALE = 1.0 / np.sqrt(128.0)
EPS = 1e-5
NEG = -1.0e9
NT = S // 128  # 16 token sub-tiles
NHB = S // 256  # 8 half-blocks of 256 tokens
NG = 4  # query-tile groups of 512 tokens
LC = L // 128  # 4 latent chunks
DC = D // 128  # 16 feature chunks

_CACHE = {}
LAST = {}


def _build(has_down_bias, has_ln_affine, has_up_bias, paired=True):
    import contextlib

    import concourse.bass as bass
    import concourse.tile as tile
    from concourse import bacc, mybir
    from concourse.masks import make_identity

    dt = mybir.dt
    f32 = dt.float32
    f32r = dt.float32r
    bf = dt.bfloat16
    ACT = mybir.ActivationFunctionType

    nc = bacc.Bacc("TRN2", target_bir_lowering=False, debug=False, num_devices=8)

    def din(name, shape, dtype=None):
        return nc.dram_tensor(
            name, shape, dtype or f32, kind="ExternalInput"
        ).ap()

    x_d = din("x", [S, D], bf)
    kbias_d = din("kbias", [128, NT])
    wqd_d = din("wqd", [D, L], bf)
    wkvd_d = din("wkvd", [D, L], bf)
    wqu_d = din("wqu", [L, HPC * HD], bf)
    wku_d = din("wku", [L, HPC * HD], bf)
    wvu_d = din("wvu", [L, HPC * HD], bf)
    wo_d = din("wo", [HPC * HD, D], bf)
    if has_down_bias:
        bqd_d = din("bqd", [1, L])
        bkvd_d = din("bkvd", [1, L])
    if has_ln_affine:
        gq_d = din("gq", [1, L])
        bq_d = din("bq", [1, L])
        gkv_d = din("gkv", [1, L])
        bkv_d = din("bkv", [1, L])
    if has_up_bias:
        bqu_d = din("bqu", [128, HPC])  # pre-scaled by SCALE on host
        bku_d = din("bku", [128, HPC])
        bvu_d = din("bvu", [1, HPC * HD])
    out_d = nc.dram_tensor("out", [S, D], f32, kind="ExternalOutput").ap()

    def r(ap):
        return ap.bitcast(f32r)

    with tile.TileContext(nc) as tc:
        with contextlib.ExitStack() as ctx:
            ctx.enter_context(
                nc.allow_low_precision(reason="float32r rounding is intentional")
            )
            const = ctx.enter_context(tc.tile_pool(name="const", bufs=1))

            ident = const.tile([128, 128], bf, tag="ident")
            ones_sq = const.tile([128, 128], f32r, tag="ones_sq")
            with tc.tile_pool(name="tmpconst", bufs=1) as tmpc:
                ident_f = tmpc.tile([128, 128], f32, tag="ident_f")
                make_identity(nc, ident_f[:])
                nc.vector.tensor_copy(ident[:], ident_f[:])
                ones_f = tmpc.tile([128, 128], f32, tag="ones_f")
                nc.gpsimd.memset(ones_f[:], 1.0)
                nc.vector.tensor_copy(ones_sq[:], ones_f[:])

            eps_col = const.tile([128, 1], f32, tag="eps_col")
            nc.gpsimd.memset(eps_col[:], EPS)
            kbias = const.tile([128, NT], f32, tag="kbias")
            nc.sync.dma_start(kbias[:], kbias_d[:])

            if has_ln_affine:
                reps = {}
                for nm, dap in (
                    ("gq", gq_d),
                    ("bq", bq_d),
                    ("gkv", gkv_d),
                    ("bkv", bkv_d),
                ):
                    t = const.tile([128, L], f32, tag=f"rep_{nm}")
                    nc.sync.dma_start(t[:], dap.broadcast_to((128, L)))
                    reps[nm] = t
            if has_down_bias:
                bd_reps = {}
                for nm, dap in (("bqd", bqd_d), ("bkvd", bkvd_d)):
                    t = const.tile([128, L], f32, tag=f"rep_{nm}")
                    nc.sync.dma_start(t[:], dap.broadcast_to((128, L)))
                    bd_reps[nm] = t
            if has_up_bias:
                bqu_sb = const.tile([128, HPC], f32, tag="bqu")
                nc.sync.dma_start(bqu_sb[:], bqu_d[:])
                bku_sb = const.tile([128, HPC], f32, tag="bku")
                nc.sync.dma_start(bku_sb[:], bku_d[:])
                bvu_rep = const.tile([128, HPC * HD], f32, tag="bvu_rep")
                nc.sync.dma_start(bvu_rep[:], bvu_d.broadcast_to((128, HPC * HD)))

            # persistent transposed latents: [128, S] per L-chunk (bf16)
            latp = ctx.enter_context(tc.tile_pool(name="latT", bufs=1))
            q_latT = [latp.tile([128, S], bf, tag=f"qlat{c}", name=f"qlat{c}") for c in range(LC)]
            kv_latT = [latp.tile([128, S], bf, tag=f"kvlat{c}", name=f"kvlat{c}") for c in range(LC)]

            # phase-2/3 persistent tiles + up-proj weights created before the
            # phase-1 pools (LIFO pool stack) so the upw DMAs can prefetch
            # during phase 1
            kqv = ctx.enter_context(tc.tile_pool(name="kqv", bufs=1))
            w = HPC * HD
            qT = [kqv.tile([128, S], bf, tag=f"qT{h}", name=f"qT{h}") for h in range(HPC)]
            kT = [kqv.tile([128, S], bf, tag=f"kT{h}", name=f"kT{h}") for h in range(HPC)]
            vtiles = [kqv.tile([128, w], bf, tag=f"vt{s}", name=f"vt{s}") for s in range(NT)]
            p2 = ctx.enter_context(contextlib.ExitStack())
            upw = p2.enter_context(tc.tile_pool(name="upw", bufs=1))
            wqu_sb = upw.tile([128, LC * w], bf, tag="wqu")
            wku_sb = upw.tile([128, LC * w], bf, tag="wku")
            wvu_sb = upw.tile([128, LC * w], bf, tag="wvu")

            p1 = ctx.enter_context(contextlib.ExitStack())
            wpool = p1.enter_context(tc.tile_pool(name="wdown", bufs=1))
            xtpool = p1.enter_context(tc.tile_pool(name="xT", bufs=24))
            zpool = p1.enter_context(tc.tile_pool(name="zpsum", bufs=2, space="PSUM"))
            tpsum = p1.enter_context(tc.tile_pool(name="tpsum", bufs=2, space="PSUM"))
            latsb = p1.enter_context(tc.tile_pool(name="latsb", bufs=4))
            stats = p1.enter_context(tc.tile_pool(name="stats", bufs=8))

            # x arrives already transposed via XBAR transpose-DMA (bf16):
            # prefetch the first 512-token slab before the weight DMAs
            NSL = S // 512
            xT_slab = {}

            def load_xT(sl):
                ts = sl * 512
                xT_slab[sl] = []
                for c in range(DC):
                    xt = xtpool.tile([128, 512], bf, tag="xT")
                    nc.sync.dma_start(
                        xt[:],
                        x_d[ts : ts + 512, c * 128 : (c + 1) * 128],
                        transpose=True,
                    )
                    xT_slab[sl].append(xt)

            load_xT(0)

            # down-proj weights resident: [128, L] per d-chunk
            wqd = []
            wkvd = []
            for c in range(DC):
                tq = wpool.tile([128, L], bf, tag=f"wqd{c}")
                nc.sync.dma_start(tq[:], wqd_d[c * 128 : (c + 1) * 128, :])
                wqd.append(tq)
                tk = wpool.tile([128, L], bf, tag=f"wkvd{c}")
                nc.sync.dma_start(tk[:], wkvd_d[c * 128 : (c + 1) * 128, :])
                wkvd.append(tk)
            # prefetch up-proj weights (used in phase 2)
            for c in range(LC):
                nc.sync.dma_start(
                    wqu_sb[:, c * w : (c + 1) * w], wqu_d[c * 128 : (c + 1) * 128, :]
                )
                nc.sync.dma_start(
                    wku_sb[:, c * w : (c + 1) * w], wku_d[c * 128 : (c + 1) * 128, :]
                )
                nc.sync.dma_start(
                    wvu_sb[:, c * w : (c + 1) * w], wvu_d[c * 128 : (c + 1) * 128, :]
                )

            # ------------- Phase 1: xT -> z -> LN -> latT ------------------
            for sl in range(NSL):
                xT = xT_slab.pop(sl)
                if sl + 1 < NSL:
                    load_xT(sl + 1)
                for s in range(4):
                    zq = zpool.tile([128, L], f32, tag="zq")
                    zkv = zpool.tile([128, L], f32, tag="zkv")
                    for c in range(DC):
                        lhs = xT[c][:, s * 128 : (s + 1) * 128]
                        nc.tensor.matmul(
                            zq[:], lhs, wqd[c][:], start=(c == 0), stop=(c == DC - 1)
                        )
                        nc.tensor.matmul(
                            zkv[:],
                            lhs,
                            wkvd[c][:],
                            start=(c == 0),
                            stop=(c == DC - 1),
                        )
                    for path, zp in (("q", zq), ("kv", zkv)):
                        if has_down_bias:
                            zsb = latsb.tile([128, L], f32, tag="zsb")
                            nc.vector.tensor_add(
                                zsb[:],
                                zp[:],
                                bd_reps["bqd" if path == "q" else "bkvd"][:],
                            )
                            zsrc = zsb
                        else:
                            zsrc = zp
                        st6 = stats.tile([128, 6], f32, tag="st6")
                        nc.vector.bn_stats(st6[:], zsrc[:])
                        mv = stats.tile([128, 2], f32, tag="mv")
                        nc.vector.bn_aggr(mv[:], st6[:])
                        mean = mv[:, 0:1]
                        var = mv[:, 1:2]
                        sq = stats.tile([128, 1], f32, tag="sq")
                        nc.scalar.activation(sq[:], var, ACT.Sqrt, bias=eps_col[:], scale=1.0)
                        r0 = stats.tile([128, 1], f32, tag="r0")
                        nc.vector.reciprocal_approx_fast(r0[:], sq[:])
                        u = stats.tile([128, 1], f32, tag="u")
                        nc.vector.tensor_mul(u[:], sq[:], r0[:])
                        u2 = stats.tile([128, 1], f32, tag="u2")
                        nc.vector.tensor_mul(u2[:], u[:], u[:])
                        t3 = stats.tile([128, 1], f32, tag="t3")
                        nc.scalar.activation(t3[:], u2[:], ACT.Copy, bias=1.5, scale=-0.5)
                        rr = stats.tile([128, 1], f32, tag="rr")
                        nc.vector.tensor_mul(rr[:], r0[:], t3[:])
                        nmr = stats.tile([128, 1], f32, tag="nmr")
                        nc.vector.tensor_mul(nmr[:], mean, rr[:])
                        nmr2 = stats.tile([128, 1], f32, tag="nmr2")
                        nc.vector.tensor_scalar_mul(nmr2[:], nmr[:], -1.0)
                        lat = latsb.tile([128, L], bf, tag="lat")
                        nc.scalar.activation(
                            lat[:], zsrc[:], ACT.Identity, bias=nmr2[:], scale=rr[:]
                        )
                        if has_ln_affine:
                            g_t = reps["gq" if path == "q" else "gkv"]
                            b_t = reps["bq" if path == "q" else "bkv"]
                            lat2 = latsb.tile([128, L], bf, tag="lat2")
                            nc.vector.tensor_mul(lat2[:], lat[:], g_t[:])
                            lat3 = latsb.tile([128, L], bf, tag="lat3")
                            nc.vector.tensor_add(lat3[:], lat2[:], b_t[:])
                            lat = lat3
                        dst = q_latT if path == "q" else kv_latT
                        pt = tpsum.tile([128, 512], bf, tag="tpl")
                        for c in range(LC):
                            nc.tensor.transpose(
                                pt[:, c * 128 : (c + 1) * 128],
                                lat[:, c * 128 : (c + 1) * 128],
                                ident[:],
                            )
                        tok0 = sl * 512 + s * 128
                        for c in range(LC):
                            dsub2 = dst[c][:, tok0 : tok0 + 128]
                            psrc = pt[:, c * 128 : (c + 1) * 128]
                            if c % 2 == 0:
                                nc.scalar.copy(dsub2, psrc)
                            else:
                                nc.vector.tensor_copy(dsub2, psrc)

            # ------------- Phase 2: up-projections -------------------------
            p1.close()
            uppsum = p2.enter_context(tc.tile_pool(name="uppsum", bufs=4, space="PSUM"))

            for G in range(NG):
                for h in range(HPC):
                    for which, wsb, dstT in (("q", wqu_sb, qT), ("k", wku_sb, kT)):
                        pp = uppsum.tile([128, 512], f32, tag="up")
                        for c in range(LC):
                            nc.tensor.matmul(
                                pp[:],
                                wsb[:, c * w + h * HD : c * w + (h + 1) * HD],
                                (q_latT if which == "q" else kv_latT)[c][
                                    :, G * 512 : (G + 1) * 512
                                ],
                                start=(c == 0),
                                stop=(c == LC - 1),
                            )
                        dsub = dstT[h][:, G * 512 : (G + 1) * 512]
                        if has_up_bias:
                            bcol = (bqu_sb if which == "q" else bku_sb)[:, h : h + 1]
                            nc.scalar.activation(
                                dsub,
                                pp[:],
                                ACT.Identity,
                                bias=bcol,
                                scale=SCALE if which == "q" else 1.0,
                            )
                        else:
                            nc.scalar.activation(
                                dsub,
                                pp[:],
                                ACT.Copy,
                                bias=0.0,
                                scale=SCALE if which == "q" else 1.0,
                            )
            for s in range(NT):
                pp = uppsum.tile([128, 512], f32, tag="up")
                for c in range(LC):
                    nc.tensor.matmul(
                        pp[:],
                        kv_latT[c][:, s * 128 : (s + 1) * 128],
                        wvu_sb[:, c * w : (c + 1) * w],
                        start=(c == 0),
                        stop=(c == LC - 1),
                    )
                if has_up_bias:
                    nc.vector.tensor_add(vtiles[s][:], pp[:], bvu_rep[:])
                else:
                    nc.vector.tensor_copy(vtiles[s][:], pp[:])

            # ------------- Phase 3: attention + out-proj -------------------
            p2.close()
            # out-proj weights resident: 4MB, loaded once while G=0
            # attention runs (frees 16MB of re-fetch + G-boundary stalls).
            # The latent tiles are dead after phase 2 — reuse them as storage.
            wo_res = q_latT[:HPC]
            for h in range(HPC):
                nc.sync.dma_start(wo_res[h][:], wo_d[h * 128 : (h + 1) * 128, :])
            maskp = ctx.enter_context(tc.tile_pool(name="maskp", bufs=1))
            cmask = maskp.tile([128, 128], f32, tag="cmask")
            nc.gpsimd.memset(cmask[:], 0.0)
            # sT[k, t]: keep 0 where (t - k) >= 0, fill NEG where k > t
            nc.gpsimd.affine_select(
                out=cmask[:],
                in_=cmask[:],
                compare_op=mybir.AluOpType.is_ge,
                fill=NEG,
                base=0,
                pattern=[[1, 128]],
                channel_multiplier=-1,
            )
            zeros_r = maskp.tile([128, 384], bf, tag="zeros_r")
            with tc.tile_pool(name="tmpz", bufs=1) as tmpz:
                zf = tmpz.tile([128, 384], f32, tag="zf")
                nc.gpsimd.memset(zf[:], 0.0)
                nc.vector.tensor_copy(zeros_r[:], zf[:])

            spsum = ctx.enter_context(tc.tile_pool(name="spsum", bufs=2, space="PSUM"))
            opsum = ctx.enter_context(tc.tile_pool(name="opsum", bufs=2, space="PSUM"))
            dpsum = ctx.enter_context(tc.tile_pool(name="dpsum", bufs=2, space="PSUM"))
            fpsum = ctx.enter_context(tc.tile_pool(name="fpsum", bufs=2, space="PSUM"))
            expp = ctx.enter_context(tc.tile_pool(name="expp", bufs=3))
            onorm = ctx.enter_context(tc.tile_pool(name="onorm", bufs=5))
            small = ctx.enter_context(tc.tile_pool(name="small", bufs=3))
            outsb = ctx.enter_context(tc.tile_pool(name="outsb", bufs=2))
            dsum = ctx.enter_context(tc.tile_pool(name="dsum", bufs=3))

            def scores_mm(dst, h, G, kc):
                nc.tensor.matmul(
                    dst,
                    kT[h][:, kc * 128 : (kc + 1) * 128],
                    qT[h][:, G * 512 : (G + 1) * 512],
                    start=True,
                    stop=True,
                )

            def av_mm(otp, es_half, h, kc, nkc):
                nc.tensor.matmul(
                    otp[:],
                    vtiles[kc][:, h * HD : (h + 1) * HD],
                    es_half,
                    start=(kc == 0),
                    stop=(kc == nkc - 1),
                )

            for G in range(NG):
                nkc = 4 * G + 4
                # exp-sums accumulate on VectorE per head (frees ~160 PE
                # matmuls). The partition reduction uses an all-ones 128x128
                # stationary, so its [128,512] output IS the broadcast of the
                # denominator to every partition: reciprocal runs full-width
                # on VectorE and the whole normalize tail stays off the PE.
                otn = []
                for h in range(HPC):
                    otp = fpsum.tile([128, 512], f32, tag="ot")
                    dacc = dsum.tile([128, 512], f32r, tag="dacc")
                    for kc in range(nkc):
                        j = kc - 4 * G
                        sp = spsum.tile([128, 512], f32, tag="sc")
                        es = expp.tile([128, 512], bf, tag="es")
                        if j <= 0:
                            # full-width key chunk (at/below the diagonal)
                            scores_mm(sp[:], h, G, kc)
                        else:
                            # diagonal band: queries < j*128 are fully masked
                            nc.tensor.matmul(
                                sp[:, j * 128 :],
                                kT[h][:, kc * 128 : (kc + 1) * 128],
                                qT[h][:, G * 512 + j * 128 : (G + 1) * 512],
                                start=True,
                                stop=True,
                            )
                        if j < 0:
                            if paired:
                                nc.scalar.activation(
                                    es[:], sp[:], ACT.Exp, bias=0.0, scale=1.0
                                )
                            else:
                                nc.scalar.activation(
                                    es[:],
                                    sp[:],
                                    ACT.Exp,
                                    bias=kbias[:, kc : kc + 1],
                                    scale=1.0,
                                )
                        else:
                            # causal mask on block j, zeros on dead columns
                            dsub = slice(j * 128, (j + 1) * 128)
                            nc.vector.tensor_add(sp[:, dsub], sp[:, dsub], cmask[:])
                            if j > 0:
                                nc.vector.tensor_copy(
                                    es[:, : j * 128], zeros_r[:, : j * 128]
                                )
                            nc.scalar.activation(
                                es[:, j * 128 :],
                                sp[:, j * 128 :],
                                ACT.Exp,
                                bias=kbias[:, kc : kc + 1],
                                scale=1.0,
                            )
                        av_mm(otp, es[:], h, kc, nkc)
                        if kc == 0:
                            nc.vector.tensor_copy(dacc[:], es[:])
                        else:
                            nc.vector.tensor_add(dacc[:], dacc[:], es[:])
                    denb = dpsum.tile([128, 512], f32, tag="denb")
                    nc.tensor.matmul(
                        denb[:], r(ones_sq[:]), r(dacc[:]), start=True, stop=True
                    )
                    rep = small.tile([128, 512], f32, tag="rep")
                    nc.vector.reciprocal_approx_fast(rep[:], denb[:])
                    ot = onorm.tile([128, 512], bf, tag="otn")
                    nc.vector.tensor_mul(ot[:], otp[:], rep[:])
                    otn.append(ot)

                for jc in range(4):
                    for ls in range(4):
                        op = opsum.tile([128, 512], f32, tag="op")
                        for h in range(HPC):
                            nc.tensor.matmul(
                                op[:],
                                otn[h][:, ls * 128 : (ls + 1) * 128],
                                wo_res[h][:, jc * 512 : (jc + 1) * 512],
                                start=(h == 0),
                                stop=(h == HPC - 1),
                            )
                        ob = outsb.tile([128, 512], f32, tag="ob")
                        if ls % 2 == 0:
                            nc.scalar.copy(ob[:], op[:])
                        else:
                            nc.vector.tensor_copy(ob[:], op[:])
                        tok0 = G * 512 + ls * 128
                        nc.sync.dma_start(
                            out_d[tok0 : tok0 + 128, jc * 512 : (jc + 1) * 512], ob[:]
                        )

    nc.compile()
    return nc


def kernel(**inputs):
    from concourse.bass_utils import run_bass_kernel_spmd

    x = np.asarray(inputs["x"], np.float32)
    mask = np.asarray(inputs["mask"])
    wq_down = np.ascontiguousarray(np.asarray(inputs["wq_down"], np.float32))
    bq_down = np.asarray(inputs["bq_down"], np.float32)
    gq_ln = np.asarray(inputs["gq_ln"], np.float32)
    bq_ln = np.asarray(inputs["bq_ln"], np.float32)
    wq_up = np.asarray(inputs["wq_up"], np.float32)
    bq_up = np.asarray(inputs["bq_up"], np.float32)
    wkv_down = np.ascontiguousarray(np.asarray(inputs["wkv_down"], np.float32))
    bkv_down = np.asarray(inputs["bkv_down"], np.float32)
    gkv_ln = np.asarray(inputs["gkv_ln"], np.float32)
    bkv_ln = np.asarray(inputs["bkv_ln"], np.float32)
    wkv_up = np.asarray(inputs["wkv_up"], np.float32)
    bkv_up = np.asarray(inputs["bkv_up"], np.float32)
    w_out = np.asarray(inputs["w_out"], np.float32)
    b_out = np.asarray(inputs["b_out"], np.float32)

    has_down_bias = bool(np.any(bq_down) or np.any(bkv_down))
    has_ln_affine = bool(
        np.any(gq_ln != 1.0) or np.any(bq_ln) or np.any(gkv_ln != 1.0) or np.any(bkv_ln)
    )
    has_up_bias = bool(np.any(bq_up) or np.any(bkv_up))
    paired = not bool(np.any(mask))
    key = (has_down_bias, has_ln_affine, has_up_bias, paired)
    if key not in _CACHE:
        _CACHE[key] = _build(*key)
    nc = _CACHE[key]

    wk_up = wkv_up[:, :D]
    wv_up = wkv_up[:, D:]
    bk_up = bkv_up[:D]
    bv_up = bkv_up[D:]

    import ml_dtypes

    BF16 = ml_dtypes.bfloat16
    xb = [np.ascontiguousarray(x[b]).astype(BF16) for b in range(B)]
    wqd_b = wq_down.astype(BF16)
    wkvd_b = wkv_down.astype(BF16)

    in_maps = []
    for core in range(NCORES):
        b = core // 4
        g = core % 4
        hs = slice(g * HPC * HD, (g + 1) * HPC * HD)
        kb = np.where(mask[b], np.float32(NEG), np.float32(0.0)).astype(np.float32)
        m = {
            "x": xb[b],
            "kbias": np.ascontiguousarray(kb.reshape(NT, 128).T),
            "wqd": wqd_b,
            "wkvd": wkvd_b,
            "wqu": np.ascontiguousarray(wq_up[:, hs]).astype(BF16),
            "wku": np.ascontiguousarray(wk_up[:, hs]).astype(BF16),
            "wvu": np.ascontiguousarray(wv_up[:, hs]).astype(BF16),
            "wo": np.ascontiguousarray(w_out[hs, :]).astype(BF16),
        }
        if has_down_bias:
            m["bqd"] = bq_down.reshape(1, L).copy()
            m["bkvd"] = bkv_down.reshape(1, L).copy()
        if has_ln_affine:
            m["gq"] = gq_ln.reshape(1, L).copy()
            m["bq"] = bq_ln.reshape(1, L).copy()
            m["gkv"] = gkv_ln.reshape(1, L).copy()
            m["bkv"] = bkv_ln.reshape(1, L).copy()
        if has_up_bias:
            m["bqu"] = np.ascontiguousarray(
                (bq_up[hs] * SCALE).reshape(HPC, 128).T.astype(np.float32)
            )
            m["bku"] = np.ascontiguousarray(bk_up[hs].reshape(HPC, 128).T)
            m["bvu"] = np.ascontiguousarray(bv_up[hs].reshape(1, HPC * HD))
        in_maps.append(m)

    import os

    trace = bool(os.environ.get("MLA_TRACE"))
    res = run_bass_kernel_spmd(
        nc, in_maps, core_ids=list(range(NCORES)), trace=trace
    )
    LAST["res"] = res
    partials = np.stack([res.results[i]["out"] for i in range(NCORES)])
    out = partials.reshape(B, 4, S, D).sum(axis=1) + b_out
    return out.astype(np.float32)

